# revision 6
# baseline (speedup 1.0000x reference)
"""FlowAttention kernel for 8 TRN2 NeuronCores.

Sharding: head-parallel. Core c owns heads {2c, 2c+1} of 16, i.e. embed
columns [128c, 128(c+1)).  Each core:
  - projects q/k/v for its 2 heads in transposed layout [dh, T] per batch
    (contraction over E with host-pre-transposed x^T, so no on-chip
    transpose of activations is needed),
  - pass A: scores[t,s] = q^T k, unnormalized exp with fused row-sum
    (ACT accum_out), normalize, write attn_weights slice (the dominant
    64 MB/core output),
  - pass B: scores^T[s,t] recomputed by swapping matmul operands, exp,
    then attn^T = v_ext^T @ exp(scores^T) where v_ext carries a fused
    ones-column producing the softmax denominator in the transposed
    layout needed for scaling,
  - out-proj partial = attn^T^T @ Wo[:, slice]^T, host sums partials.
"""

import numpy as np

import concourse.bass as bass
import concourse.tile as tile
from concourse import bacc, mybir
from concourse.bass import ts
from concourse.bass_utils import run_bass_kernel_spmd
from concourse.masks import make_identity

# Problem constants (hardcoded per harness contract).
T = 2048          # sequence length
B = 2             # batch
E = 1024          # embed dim
H = 16            # heads
DH = 64           # head dim
N_CORES = 8
EC = E // N_CORES  # embed cols per core (= 2 heads * DH = 128)
SCALING = DH ** -0.5
EXP_BIAS = -10.0   # constant shift inside exp; cancels in softmax

f32 = mybir.dt.float32
f32r = mybir.dt.float32r
bf16 = mybir.dt.bfloat16
Exp = mybir.ActivationFunctionType.Exp


def r(ap):
    """View an fp32 AP as float32r for full-rate PE matmuls."""
    return ap.bitcast(f32r)


def build_kernel(t=T):
    tt_n = t // 128    # 128-tiles along t/s
    q_n = t // 512     # 512-chunks along t/s
    nc = bacc.Bacc("TRN2", target_bir_lowering=False)

    # ---- DRAM I/O (per-core shapes) ----
    xt = nc.dram_tensor("xt", [B, E, t], f32r, kind="ExternalInput")      # x^T per batch
    wq = nc.dram_tensor("wq", [E, EC], f32r, kind="ExternalInput")        # (s*Wq[sl]).T
    wk = nc.dram_tensor("wk", [E, EC], f32r, kind="ExternalInput")
    wv = nc.dram_tensor("wv", [E, EC], f32r, kind="ExternalInput")
    bq = nc.dram_tensor("bq", [EC, t], f32, kind="ExternalInput")        # (s*bias_q[:,sl]).T
    bk = nc.dram_tensor("bk", [EC, t], f32, kind="ExternalInput")
    bv = nc.dram_tensor("bv", [EC, t], f32, kind="ExternalInput")
    wo0 = nc.dram_tensor("wo0", [DH, E], f32r, kind="ExternalInput")      # Wo[:, sl j=0].T
    wo1 = nc.dram_tensor("wo1", [DH, E], f32r, kind="ExternalInput")
    wout = nc.dram_tensor("wout", [B, 2, t, t], f32, kind="ExternalOutput")
    pout = nc.dram_tensor("pout", [B, t, E], f32, kind="ExternalOutput")

    with tile.TileContext(nc) as tc:
        with tc.tile_pool(name="persist", bufs=1) as pers:
            # Persistent SBUF: projections + v_ext + out-proj weights.
            qT = [pers.tile([128, t], f32r, name=f"qT{b}", tag=f"qT{b}") for b in range(B)]
            kT = [pers.tile([128, t], f32r, name=f"kT{b}", tag=f"kT{b}") for b in range(B)]
            # v_ext[:, st, 0:64]=v_j0, 64: ones, 65:129=v_j1, 129: ones
            v_ext = [pers.tile([128, tt_n, 130], bf16, name=f"vx{b}", tag=f"vx{b}")
                     for b in range(B)]
            wo_sb = [pers.tile([DH, E], f32r, name=f"wo{j}", tag=f"wo{j}") for j in range(2)]
            nc.sync.dma_start(out=wo_sb[0], in_=wo0[:, :])
            nc.sync.dma_start(out=wo_sb[1], in_=wo1[:, :])
            exp_bias = pers.tile([128, 1], f32)
            nc.vector.memset(exp_bias, EXP_BIAS)
            for b in range(B):
                nc.vector.memset(v_ext[b][:, :, 64:65], 1.0)
                nc.vector.memset(v_ext[b][:, :, 129:130], 1.0)

            # ---------------- Stage P: projections ----------------
            with tc.tile_pool(name="pin", bufs=1) as pin, \
                 tc.tile_pool(name="pwork", bufs=2) as pwork, \
                 tc.tile_pool(name="ppsum", bufs=2, space="PSUM") as ppsum, \
                 tc.tile_pool(name="tpsum", bufs=2, space="PSUM") as tpsum:
                identity = pin.tile([128, 128], f32)
                make_identity(nc, identity)
                wq_sb = pin.tile([128, 8, EC], f32r)
                wk_sb = pin.tile([128, 8, EC], f32r)
                wv_sb = pin.tile([128, 8, EC], f32r)
                nc.sync.dma_start(out=wq_sb, in_=wq[:, :].rearrange("(k p) m -> p k m", p=128))
                nc.sync.dma_start(out=wk_sb, in_=wk[:, :].rearrange("(k p) m -> p k m", p=128))
                nc.sync.dma_start(out=wv_sb, in_=wv[:, :].rearrange("(k p) m -> p k m", p=128))
                bq_sb = pin.tile([128, t], f32)
                bk_sb = pin.tile([128, t], f32)
                bv_sb = pin.tile([128, t], f32)
                nc.sync.dma_start(out=bq_sb, in_=bq[:, :])
                nc.sync.dma_start(out=bk_sb, in_=bk[:, :])
                nc.sync.dma_start(out=bv_sb, in_=bv[:, :])

                for b in range(B):
                    xt_sb = pwork.tile([128, 8, t], f32r, tag="xt", bufs=1)
                    for k in range(8):
                        nc.sync.dma_start(
                            out=xt_sb[:, k, :], in_=xt[b, ts(k, 128), :])
                    vT = pwork.tile([128, t], f32, tag="vT", bufs=1)
                    for (w_sb, b_sb, dst) in (
                        (wq_sb, bq_sb, qT[b]),
                        (wk_sb, bk_sb, kT[b]),
                        (wv_sb, bv_sb, vT),
                    ):
                        for c4 in range(q_n):
                            ps = ppsum.tile([128, 512], f32, tag="proj")
                            for k in range(8):
                                nc.tensor.matmul(
                                    ps, lhsT=w_sb[:, k, :],
                                    rhs=xt_sb[:, k, ts(c4, 512)],
                                    start=(k == 0), stop=(k == 7))
                            nc.vector.tensor_add(
                                out=dst[:, ts(c4, 512)], in0=ps,
                                in1=b_sb[:, ts(c4, 512)])
                    # transpose v^T -> v natural, fused into v_ext (bf16)
                    for st in range(tt_n):
                        pt = tpsum.tile([128, 128], f32, tag="tr")
                        nc.tensor.transpose(pt, vT[:, ts(st, 128)], identity)
                        nc.vector.tensor_copy(out=v_ext[b][:, st, 0:64], in_=pt[:, 0:64])
                        nc.vector.tensor_copy(out=v_ext[b][:, st, 65:129], in_=pt[:, 64:128])

            # ---------------- Stages A (scores+softmax), B (attn^T), C (out-proj) ----
            with tc.tile_pool(name="apsum", bufs=1, space="PSUM") as apsum, \
                 tc.tile_pool(name="btpsum", bufs=2, space="PSUM") as btpsum, \
                 tc.tile_pool(name="avpsum", bufs=2, space="PSUM") as avpsum, \
                 tc.tile_pool(name="cpsum", bufs=1, space="PSUM") as cpsum, \
                 tc.tile_pool(name="awork", bufs=3) as awork, \
                 tc.tile_pool(name="stats", bufs=6) as stats, \
                 tc.tile_pool(name="attnp", bufs=1) as attnp, \
                 tc.tile_pool(name="cwork", bufs=2) as cwork:
                for b in range(B):
                    attnT = [attnp.tile([DH, t], f32r, name=f"attnT{b}{j}", tag=f"attnT{j}")
                             for j in range(2)]
                    for j in range(2):
                        qj = qT[b][64 * j:64 * (j + 1), :]
                        kj = kT[b][64 * j:64 * (j + 1), :]
                        # ---- Pass A: scores[t,s], softmax, weights out ----
                        a_ch = min(1024, t)       # psS chunk width
                        n_ach = t // a_ch
                        for tt in range(tt_n):
                            wA = awork.tile([128, t], f32, tag="wA")
                            zs = stats.tile([128, n_ach], f32, tag="z")
                            for hh in range(n_ach):
                                psS = apsum.tile([128, a_ch], f32, tag="scores")
                                for sc in range(a_ch // 512):
                                    nc.tensor.matmul(
                                        psS[:, ts(sc, 512)],
                                        lhsT=qj[:, ts(tt, 128)],
                                        rhs=kj[:, ts(hh * (a_ch // 512) + sc, 512)])
                                nc.scalar.activation(
                                    wA[:, ts(hh, a_ch)], psS, Exp,
                                    bias=exp_bias, scale=1.0,
                                    accum_out=zs[:, hh:hh + 1])
                            if n_ach == 2:
                                z = stats.tile([128, 1], f32, tag="zt")
                                nc.vector.tensor_add(out=z, in0=zs[:, 0:1], in1=zs[:, 1:2])
                            else:
                                z = zs[:, 0:1]
                            rz = stats.tile([128, 1], f32, tag="rz")
                            nc.vector.reciprocal(rz, z)
                            nc.vector.tensor_scalar_mul(wA, in0=wA, scalar1=rz)
                            nc.sync.dma_start(out=wout[b, j, ts(tt, 128), :], in_=wA)
                        # ---- Pass B: scores^T[s,t], attn^T accumulate ----
                        for h in range(q_n):
                            psAV = avpsum.tile([65, 512], f32, tag="av")
                            for st in range(tt_n):
                                psT = btpsum.tile([128, 512], f32, tag="sT")
                                nc.tensor.matmul(
                                    psT, lhsT=kj[:, ts(st, 128)],
                                    rhs=qj[:, ts(h, 512)])
                                wT = awork.tile([128, 512], bf16, tag="wT")
                                nc.scalar.activation(wT, psT, Exp, bias=exp_bias, scale=1.0)
                                nc.tensor.matmul(
                                    psAV, lhsT=v_ext[b][:, st, 65 * j:65 * (j + 1)],
                                    rhs=wT, start=(st == 0), stop=(st == tt_n - 1))
                            rzT = stats.tile([1, 512], f32, tag="rzT")
                            nc.vector.reciprocal(rzT, psAV[64:65, :])
                            scq = stats.tile([DH, 512], f32, tag="scq")
                            nc.gpsimd.partition_broadcast(scq, rzT)
                            nc.vector.tensor_mul(
                                out=attnT[j][:, ts(h, 512)], in0=psAV[0:64, :], in1=scq)
                    # ---- Stage C: out-proj partials for this batch ----
                    for tt in range(tt_n):
                        po = cwork.tile([128, E], f32, tag="po")
                        psO = cpsum.tile([128, E], f32, tag="o")
                        for ech in range(2):
                            nc.tensor.matmul(
                                psO[:, ts(ech, 512)],
                                lhsT=attnT[0][:, ts(tt, 128)],
                                rhs=wo_sb[0][:, ts(ech, 512)],
                                start=True, stop=False)
                            nc.tensor.matmul(
                                psO[:, ts(ech, 512)],
                                lhsT=attnT[1][:, ts(tt, 128)],
                                rhs=wo_sb[1][:, ts(ech, 512)],
                                start=False, stop=True)
                        nc.vector.tensor_copy(out=po, in_=psO)
                        nc.sync.dma_start(out=pout[b, ts(tt, 128), :], in_=po)

    nc.finalize()
    return nc


def shard_inputs(query, bias_q, bias_k, bias_v, Wq, Wk, Wv, Wo, t=T):
    """Build per-core input maps (host-side shard + layout prep)."""
    xt = np.ascontiguousarray(query.transpose(1, 2, 0)).astype(np.float32)  # [B, E, T]
    in_maps = []
    for c in range(N_CORES):
        sl = slice(EC * c, EC * (c + 1))
        m = {
            "xt": xt,
            "wq": np.ascontiguousarray((SCALING * Wq[sl, :]).T.astype(np.float32)),
            "wk": np.ascontiguousarray(Wk[sl, :].T.astype(np.float32)),
            "wv": np.ascontiguousarray(Wv[sl, :].T.astype(np.float32)),
            "bq": np.ascontiguousarray((SCALING * bias_q[:, sl]).T.astype(np.float32)),
            "bk": np.ascontiguousarray(bias_k[:, sl].T.astype(np.float32)),
            "bv": np.ascontiguousarray(bias_v[:, sl].T.astype(np.float32)),
            "wo0": np.ascontiguousarray(Wo[:, EC * c: EC * c + DH].T.astype(np.float32)),
            "wo1": np.ascontiguousarray(Wo[:, EC * c + DH: EC * (c + 1)].T.astype(np.float32)),
        }
        in_maps.append(m)
    return in_maps


def assemble_outputs(results, bo, t=T):
    attn_weights = np.empty((B * H, t, t), dtype=np.float32)
    partial = np.zeros((B, t, E), dtype=np.float32)
    for c in range(N_CORES):
        wout = results[c]["wout"]  # [B, 2, t, t]
        for b in range(B):
            for j in range(2):
                attn_weights[b * H + 2 * c + j] = wout[b, j]
        partial += results[c]["pout"]
    attn = partial.transpose(1, 0, 2) + bo[None, None, :].astype(np.float32)
    return np.ascontiguousarray(attn), attn_weights


_NC_CACHE = {}


def kernel(query, key, value, bias_q, bias_k, bias_v, Wq, Wk, Wv, Wo, bo):
    t = query.shape[0]
    if t not in _NC_CACHE:
        _NC_CACHE[t] = build_kernel(t)
    nc = _NC_CACHE[t]
    in_maps = shard_inputs(query, bias_q, bias_k, bias_v, Wq, Wk, Wv, Wo, t=t)
    res = run_bass_kernel_spmd(nc, in_maps, core_ids=list(range(N_CORES)))
    return assemble_outputs(res.results, np.asarray(bo), t=t)


# revision 28
# speedup vs baseline: 1.1222x; 1.1222x over previous
"""FlowAttention kernel for 8 TRN2 NeuronCores.

Sharding: head-parallel. Core c owns heads {2c, 2c+1} of 16, i.e. embed
columns [128c, 128(c+1)).  Each core:
  - projects q/k/v for its 2 heads in transposed layout [dh, T] per batch
    (contraction over E with host-pre-transposed x^T, so no on-chip
    transpose of activations is needed),
  - pass A: scores[t,s] = q^T k, unnormalized exp with fused row-sum
    (ACT accum_out), normalize, write attn_weights slice (the dominant
    64 MB/core output) via SWDGE bf16->f32 cast DMA,
  - pass B: scores^T[s,t] recomputed by swapping matmul operands, exp,
    then attn^T = v_ext^T @ exp(scores^T) where v_ext carries a fused
    ones-column producing the softmax denominator in the transposed
    layout needed for scaling,
  - out-proj partial = attn^T^T @ Wo[:, slice]^T, host sums partials.

PSUM budget (8 banks): "ab" tag 2x[128,1024] (4 banks) shared by pass-A
score chunks, pass-B transposed-score chunks and projection psums;
"avc" tag 4x[128,512] (4 banks) shared by pass-B quarter-accumulators,
stage-C out-proj tiles and stage-P transposes.

Emission order interleaves stage P of batch 1 under pass A of batch 0
so projection DMA/PE work fills the ACT-bound phases.
"""

import numpy as np

import concourse.bass as bass
import concourse.tile as tile
from concourse import bacc, mybir
from concourse.bass import ts
from concourse.bass_utils import run_bass_kernel_spmd
from concourse.masks import make_identity

# Problem constants (hardcoded per harness contract).
T = 2048          # sequence length
B = 2             # batch
E = 1024          # embed dim
H = 16            # heads
DH = 64           # head dim
N_CORES = 8
EC = E // N_CORES  # embed cols per core (= 2 heads * DH = 128)
SCALING = DH ** -0.5
EXP_BIAS = -10.0   # constant shift inside exp; cancels in softmax

f32 = mybir.dt.float32
f32r = mybir.dt.float32r
bf16 = mybir.dt.bfloat16
Exp = mybir.ActivationFunctionType.Exp


def build_kernel(t=T):
    tt_n = t // 128        # 128-tiles along t/s
    q_n = t // 512         # 512-chunks along t/s
    a_ch = min(1024, t)    # score-psum chunk width
    n_ach = t // a_ch
    u_n = a_ch // 512
    nc = bacc.Bacc("TRN2", target_bir_lowering=False)

    # ---- DRAM I/O (per-core shapes) ----
    xt = nc.dram_tensor("xt", [B, E, t], f32r, kind="ExternalInput")
    wq = nc.dram_tensor("wq", [E, EC], f32r, kind="ExternalInput")
    wk = nc.dram_tensor("wk", [E, EC], f32r, kind="ExternalInput")
    wv = nc.dram_tensor("wv", [E, EC], f32r, kind="ExternalInput")
    bq = nc.dram_tensor("bq", [EC, t], f32, kind="ExternalInput")
    bk = nc.dram_tensor("bk", [EC, t], f32, kind="ExternalInput")
    bv = nc.dram_tensor("bv", [EC, t], f32, kind="ExternalInput")
    wo0 = nc.dram_tensor("wo0", [DH, E], f32, kind="ExternalInput")
    wo1 = nc.dram_tensor("wo1", [DH, E], f32, kind="ExternalInput")
    wout = nc.dram_tensor("wout", [B, 2, t, t], f32, kind="ExternalOutput")
    pout = nc.dram_tensor("pout", [B, t, E], bf16, kind="ExternalOutput")

    with tile.TileContext(nc) as tc:
        pers = tc.alloc_tile_pool(name="persist", bufs=1)
        pin = tc.alloc_tile_pool(name="pin", bufs=1)
        pwork = tc.alloc_tile_pool(name="pwork", bufs=1)
        abpsum = tc.alloc_tile_pool(name="abpsum", bufs=2, space="PSUM")
        avcpsum = tc.alloc_tile_pool(name="avcpsum", bufs=4, space="PSUM")
        awork = tc.alloc_tile_pool(name="awork", bufs=6)
        bwork = tc.alloc_tile_pool(name="bwork", bufs=3)
        stats = tc.alloc_tile_pool(name="stats", bufs=4)
        bstats = tc.alloc_tile_pool(name="bstats", bufs=3)
        attnp = tc.alloc_tile_pool(name="attnp", bufs=2)
        cwork = tc.alloc_tile_pool(name="cwork", bufs=2)

        qT = [pers.tile([128, t], f32r, name=f"qT{b}", tag=f"qT{b}") for b in range(B)]
        kT = [pers.tile([128, t], f32r, name=f"kT{b}", tag=f"kT{b}") for b in range(B)]
        # v_ext cols: [0:64]=v_j0, 64=ones, [65:129]=v_j1, 129=ones
        v_ext = [pers.tile([128, tt_n, 130], bf16, name=f"vx{b}", tag=f"vx{b}")
                 for b in range(B)]
        wo_sb = [pers.tile([DH, E], bf16, name=f"wo{j}", tag=f"wo{j}") for j in range(2)]
        with tc.high_priority(offset=-150):
            nc.gpsimd.dma_start(out=wo_sb[0], in_=wo0[:, :])
            nc.gpsimd.dma_start(out=wo_sb[1], in_=wo1[:, :])
        exp_bias = pers.tile([128, 1], f32)
        nc.vector.memset(exp_bias, EXP_BIAS)
        for b in range(B):
            nc.vector.memset(v_ext[b][:, :, 64:65], 1.0)
            nc.vector.memset(v_ext[b][:, :, 129:130], 1.0)

        identity = pin.tile([128, 128], f32)
        make_identity(nc, identity)
        wq_sb = pin.tile([128, 8, EC], f32r)
        wk_sb = pin.tile([128, 8, EC], f32r)
        wv_sb = pin.tile([128, 8, EC], f32r)
        nc.sync.dma_start(out=wq_sb, in_=wq[:, :].rearrange("(k p) m -> p k m", p=128))
        nc.sync.dma_start(out=wk_sb, in_=wk[:, :].rearrange("(k p) m -> p k m", p=128))
        nc.sync.dma_start(out=wv_sb, in_=wv[:, :].rearrange("(k p) m -> p k m", p=128))
        bq_sb = pin.tile([128, t], bf16)
        bk_sb = pin.tile([128, t], bf16)
        bv_sb = pin.tile([128, t], bf16)
        with tc.high_priority(offset=-60):
            nc.gpsimd.dma_start(out=bq_sb, in_=bq[:, :])
            nc.gpsimd.dma_start(out=bk_sb, in_=bk[:, :])
            nc.gpsimd.dma_start(out=bv_sb, in_=bv[:, :])

        attnT = {}
        filler = []           # deque of (pe_cost_us, thunk)
        v_done = {0: 0, 1: 0}   # v_ext tiles emitted per batch
        p_ready = {0: True, 1: False}

        def drain_budget(budget):
            while filler and budget > 0:
                c, fn = filler.pop(0)
                fn()
                budget -= c

        def drain_all():
            drain_budget(10 ** 9)

        def need_v(b, st):
            """Force-drain until v_ext[b][:, st] has been emitted (the AV
            matmul that reads it must be emitted after the writer)."""
            while v_done[b] <= st:
                c, fn = filler.pop(0)
                fn()

        def need_p(b):
            while not p_ready[b]:
                c, fn = filler.pop(0)
                fn()

        def proj_chunk(w_sb, b_sb, dst, xt_sb, c4):
            ps = avcpsum.tile([128, 512], f32, tag="avc", name="proj")
            for k in range(8):
                nc.tensor.matmul(
                    ps, lhsT=w_sb[:, k, :], rhs=xt_sb[:, k, ts(c4, 512)],
                    start=(k == 0), stop=(k == 7))
            nc.vector.tensor_add(
                out=dst[:, ts(c4, 512)], in0=ps, in1=b_sb[:, ts(c4, 512)])

        def v_tr_chunk(b, vT, st):
            pt = avcpsum.tile([128, 128], f32, tag="avc", name="tr")
            nc.tensor.transpose(pt, vT[:, ts(st, 128)], identity)
            nc.vector.tensor_copy(out=v_ext[b][:, st, 0:64], in_=pt[:, 0:64])
            nc.vector.tensor_copy(out=v_ext[b][:, st, 65:129], in_=pt[:, 64:128])
            v_done[b] = st + 1

        def stage_p(b, defer=False):
            """Emit projections. defer=True queues everything after the xt
            loads on the filler deque; defer=False emits q/k inline (pass A
            needs them first) and defers only the v chain."""
            xt_sb = pwork.tile([128, 8, t], f32r, tag="xt", name=f"xt{b}")
            vT = pwork.tile([128, t], f32, tag="vT", name=f"vT{b}")
            for k in range(8):
                nc.sync.dma_start(out=xt_sb[:, k, :], in_=xt[b, ts(k, 128), :])
            units = []
            # q/k chunks interleaved so pass A can start earliest
            for c4 in range(q_n):
                units.append((2.2, lambda c4=c4: proj_chunk(wq_sb, bq_sb, qT[b], xt_sb, c4)))
                units.append((2.2, lambda c4=c4: proj_chunk(wk_sb, bk_sb, kT[b], xt_sb, c4)))
            if not defer:
                for _, u in units:
                    u()
                units = []
            else:
                def mark_ready():
                    p_ready[b] = True
                last_c, last_u = units[-1]
                units[-1] = (last_c, lambda: (last_u(), mark_ready())[0])
            # v chunks with their transposes right behind (tr st needs
            # vT chunk st//4 only)
            for c4 in range(q_n):
                units.append((2.2, lambda c4=c4: proj_chunk(wv_sb, bv_sb, vT, xt_sb, c4)))
                for st in range(c4 * (tt_n // q_n), (c4 + 1) * (tt_n // q_n)):
                    units.append((0.4, lambda st=st: v_tr_chunk(b, vT, st)))
            filler.extend(units)

        def emit_a_tile(b, j, tt):
            qj = qT[b][64 * j:64 * (j + 1), :]
            kj = kT[b][64 * j:64 * (j + 1), :]
            wA = awork.tile([128, t], bf16, tag="wA", name="wA")
            # Alternate the row-sum between ACT's accumulator (187-279ns
            # tax per exp) and a DVE reduce (~2.1us) to balance both engines.
            on_act = tt % 2 == 0
            zs = stats.tile([128, n_ach], f32, tag="z", name="zs")
            for hh in range(n_ach):
                psS = abpsum.tile([128, a_ch], f32, tag="ab", name="psS")
                for sc in range(u_n):
                    nc.tensor.matmul(
                        psS[:, ts(sc, 512)],
                        lhsT=qj[:, ts(tt, 128)],
                        rhs=kj[:, ts(hh * u_n + sc, 512)])
                nc.scalar.activation(
                    wA[:, ts(hh, a_ch)], psS, Exp, bias=exp_bias, scale=1.0,
                    accum_out=zs[:, hh:hh + 1] if on_act else None)
            z = stats.tile([128, 1], f32, tag="zt", name="z")
            if on_act:
                if n_ach == 2:
                    nc.vector.tensor_add(out=z, in0=zs[:, 0:1], in1=zs[:, 1:2])
                else:
                    z = zs[:, 0:1]
            else:
                nc.vector.reduce_sum(out=z, in_=wA, axis=mybir.AxisListType.X)
            rz = stats.tile([128, 1], f32, tag="rz", name="rz")
            nc.vector.reciprocal(rz, z)
            nc.vector.tensor_scalar_mul(wA, in0=wA, scalar1=rz)
            # SWDGE store casts bf16 -> f32 on the way out
            nc.gpsimd.dma_start(out=wout[b, j, ts(tt, 128), :], in_=wA)

        def c_tile(b, tt):
            a0, a1 = attnT[(b, 0)], attnT[(b, 1)]
            po = cwork.tile([128, E], bf16, tag="po", name="po")
            for ech in range(2):
                psO = avcpsum.tile([128, 512], f32, tag="avc", name="psO")
                nc.tensor.matmul(
                    psO, lhsT=a0[:, ts(tt, 128)], rhs=wo_sb[0][:, ts(ech, 512)],
                    start=True, stop=False)
                nc.tensor.matmul(
                    psO, lhsT=a1[:, ts(tt, 128)], rhs=wo_sb[1][:, ts(ech, 512)],
                    start=False, stop=True)
                nc.vector.tensor_copy(out=po[:, ts(ech, 512)], in_=psO)
            nc.sync.dma_start(out=pout[b, ts(tt, 128), :], in_=po)

        def queue_c_half(b, hh):
            for tt in range(hh * (tt_n // n_ach), (hh + 1) * (tt_n // n_ach)):
                filler.append((0.9, lambda tt=tt: c_tile(b, tt)))

        def stage_ab(b, j, budget=0.6, a_dense=False):
            """Pass B slices with pass-A tiles interleaved, so the
            weights-out DMA spreads across the whole (A+B) window. Each
            slice drains up to `budget` us of filler PE work (P/C chunks).
            a_dense packs all A tiles into the first half of the block
            (used for the final block so the tail only holds stage C)."""
            qj = qT[b][64 * j:64 * (j + 1), :]
            kj = kT[b][64 * j:64 * (j + 1), :]
            aT = attnp.tile([DH, t], bf16, name=f"attnT{b}{j}", tag=f"attnT{j}")
            attnT[(b, j)] = aT
            need_p(b)
            a_idx = 0
            for hh in range(n_ach):
                psAV = [avcpsum.tile([65, 512], f32, tag="avc", name=f"psAV{u}")
                        for u in range(u_n)]
                for st in range(tt_n):
                    need_v(b, st)
                    drain_budget(budget)
                    psT = abpsum.tile([128, a_ch], f32, tag="ab", name="psT")
                    for u in range(u_n):
                        nc.tensor.matmul(
                            psT[:, ts(u, 512)], lhsT=kj[:, ts(st, 128)],
                            rhs=qj[:, ts(hh * u_n + u, 512)])
                    wT = bwork.tile([128, a_ch], bf16, tag="wT", name="wT")
                    nc.scalar.activation(wT, psT, Exp, bias=exp_bias, scale=1.0)
                    for u in range(u_n):
                        nc.tensor.matmul(
                            psAV[u], lhsT=v_ext[b][:, st, 65 * j:65 * (j + 1)],
                            rhs=wT[:, ts(u, 512)],
                            start=(st == 0), stop=(st == tt_n - 1))
                    slice_i = hh * tt_n + st
                    a_every = 1 if a_dense else n_ach
                    if a_idx < tt_n and slice_i % a_every == 0:
                        emit_a_tile(b, j, a_idx)
                        a_idx += 1
                for u in range(u_n):
                    h = hh * u_n + u
                    rzT = bstats.tile([1, 512], f32, tag="rzT", name="rzT")
                    nc.vector.reciprocal(rzT, psAV[u][64:65, :])
                    scq = bstats.tile([DH, 512], f32, tag="scq", name="scq")
                    nc.gpsimd.partition_broadcast(scq, rzT)
                    nc.vector.tensor_mul(
                        out=aT[:, ts(h, 512)], in0=psAV[u][0:64, :], in1=scq)
                if j == 1:
                    queue_c_half(b, hh)

        # P and C chunks ride the filler deque, drained by PE-cost budget
        # per B-slice, so their PE/DMA work fills the ACT-bound stretches
        # without monopolizing the shared "avc" psum slots.
        stage_p(0)            # q/k inline; v chain deferred
        stage_p(1, defer=True)
        stage_ab(0, 0)
        stage_ab(0, 1)
        stage_ab(1, 0)
        stage_ab(1, 1, a_dense=True)
        drain_all()

        for p in (cwork, attnp, bstats, stats, bwork, awork, avcpsum, abpsum, pwork, pin, pers):
            p.release()

    nc.finalize()
    return nc


def shard_inputs(query, bias_q, bias_k, bias_v, Wq, Wk, Wv, Wo, t=T):
    """Build per-core input maps (host-side shard + layout prep)."""
    xt = np.ascontiguousarray(query.transpose(1, 2, 0)).astype(np.float32)  # [B, E, T]
    in_maps = []
    for c in range(N_CORES):
        sl = slice(EC * c, EC * (c + 1))
        m = {
            "xt": xt,
            "wq": np.ascontiguousarray((SCALING * Wq[sl, :]).T.astype(np.float32)),
            "wk": np.ascontiguousarray(Wk[sl, :].T.astype(np.float32)),
            "wv": np.ascontiguousarray(Wv[sl, :].T.astype(np.float32)),
            "bq": np.ascontiguousarray((SCALING * bias_q[:, sl]).T.astype(np.float32)),
            "bk": np.ascontiguousarray(bias_k[:, sl].T.astype(np.float32)),
            "bv": np.ascontiguousarray(bias_v[:, sl].T.astype(np.float32)),
            "wo0": np.ascontiguousarray(Wo[:, EC * c: EC * c + DH].T.astype(np.float32)),
            "wo1": np.ascontiguousarray(Wo[:, EC * c + DH: EC * (c + 1)].T.astype(np.float32)),
        }
        in_maps.append(m)
    return in_maps


def assemble_outputs(results, bo, t=T):
    attn_weights = np.empty((B * H, t, t), dtype=np.float32)
    partial = np.zeros((B, t, E), dtype=np.float32)
    for c in range(N_CORES):
        wout = results[c]["wout"]  # [B, 2, t, t]
        for b in range(B):
            for j in range(2):
                attn_weights[b * H + 2 * c + j] = wout[b, j]
        partial += results[c]["pout"]
    attn = partial.transpose(1, 0, 2) + bo[None, None, :].astype(np.float32)
    return np.ascontiguousarray(attn), attn_weights


_NC_CACHE = {}


def kernel(query, key, value, bias_q, bias_k, bias_v, Wq, Wk, Wv, Wo, bo):
    t = query.shape[0]
    if t not in _NC_CACHE:
        _NC_CACHE[t] = build_kernel(t)
    nc = _NC_CACHE[t]
    in_maps = shard_inputs(query, bias_q, bias_k, bias_v, Wq, Wk, Wv, Wo, t=t)
    res = run_bass_kernel_spmd(nc, in_maps, core_ids=list(range(N_CORES)))
    return assemble_outputs(res.results, np.asarray(bo), t=t)


# revision 34
# speedup vs baseline: 1.1361x; 1.0124x over previous
"""FlowAttention kernel for 8 TRN2 NeuronCores.

Sharding: head-parallel. Core c owns heads {2c, 2c+1} of 16, i.e. embed
columns [128c, 128(c+1)).  Each core:
  - projects q/k/v for its 2 heads in transposed layout [dh, T] per batch
    (contraction over E with host-pre-transposed x^T, so no on-chip
    transpose of activations is needed),
  - pass A: scores[t,s] = q^T k, unnormalized exp with fused row-sum
    (ACT accum_out), normalize, write attn_weights slice (the dominant
    64 MB/core output) via SWDGE bf16->f32 cast DMA,
  - pass B: scores^T[s,t] recomputed by swapping matmul operands, exp,
    then attn^T = v_ext^T @ exp(scores^T) where v_ext carries a fused
    ones-column producing the softmax denominator in the transposed
    layout needed for scaling,
  - out-proj partial = attn^T^T @ Wo[:, slice]^T, host sums partials.

PSUM budget (8 banks): "ab" tag 2x[128,1024] (4 banks) shared by pass-A
score chunks, pass-B transposed-score chunks and projection psums;
"avc" tag 4x[128,512] (4 banks) shared by pass-B quarter-accumulators,
stage-C out-proj tiles and stage-P transposes.

Emission order interleaves stage P of batch 1 under pass A of batch 0
so projection DMA/PE work fills the ACT-bound phases.
"""

import numpy as np

import concourse.bass as bass
import concourse.tile as tile
from concourse import bacc, mybir
from concourse.bass import ts
from concourse.bass_utils import run_bass_kernel_spmd
from concourse.masks import make_identity

# Problem constants (hardcoded per harness contract).
T = 2048          # sequence length
B = 2             # batch
E = 1024          # embed dim
H = 16            # heads
DH = 64           # head dim
N_CORES = 8
EC = E // N_CORES  # embed cols per core (= 2 heads * DH = 128)
SCALING = DH ** -0.5
EXP_BIAS = -10.0   # constant shift inside exp; cancels in softmax

f32 = mybir.dt.float32
f32r = mybir.dt.float32r
bf16 = mybir.dt.bfloat16
Exp = mybir.ActivationFunctionType.Exp


def build_kernel(t=T):
    tt_n = t // 128        # 128-tiles along t/s
    q_n = t // 512         # 512-chunks along t/s
    a_ch = min(1024, t)    # score-psum chunk width
    n_ach = t // a_ch
    u_n = a_ch // 512
    nc = bacc.Bacc("TRN2", target_bir_lowering=False)

    # ---- DRAM I/O (per-core shapes) ----
    xt = nc.dram_tensor("xt", [B, E, t], f32r, kind="ExternalInput")
    wq = nc.dram_tensor("wq", [E, EC], f32r, kind="ExternalInput")
    wk = nc.dram_tensor("wk", [E, EC], f32r, kind="ExternalInput")
    wv = nc.dram_tensor("wv", [E, EC], f32r, kind="ExternalInput")
    bq = nc.dram_tensor("bq", [EC, t], f32, kind="ExternalInput")
    bk = nc.dram_tensor("bk", [EC, t], f32, kind="ExternalInput")
    bv = nc.dram_tensor("bv", [EC, t], f32, kind="ExternalInput")
    wo0 = nc.dram_tensor("wo0", [DH, E], f32, kind="ExternalInput")
    wo1 = nc.dram_tensor("wo1", [DH, E], f32, kind="ExternalInput")
    wout = nc.dram_tensor("wout", [B, 2, t, t], f32, kind="ExternalOutput")
    pout = nc.dram_tensor("pout", [B, t, E], bf16, kind="ExternalOutput")

    with tile.TileContext(nc) as tc:
        pers = tc.alloc_tile_pool(name="persist", bufs=1)
        pin = tc.alloc_tile_pool(name="pin", bufs=1)
        pwork = tc.alloc_tile_pool(name="pwork", bufs=1)
        abpsum = tc.alloc_tile_pool(name="abpsum", bufs=2, space="PSUM")
        avcpsum = tc.alloc_tile_pool(name="avcpsum", bufs=4, space="PSUM")
        awork = tc.alloc_tile_pool(name="awork", bufs=7)
        bwork = tc.alloc_tile_pool(name="bwork", bufs=2)
        stats = tc.alloc_tile_pool(name="stats", bufs=4)
        bstats = tc.alloc_tile_pool(name="bstats", bufs=2)
        attnp = tc.alloc_tile_pool(name="attnp", bufs=2)
        cwork = tc.alloc_tile_pool(name="cwork", bufs=2)

        qT = [pers.tile([128, t], f32r, name=f"qT{b}", tag=f"qT{b}") for b in range(B)]
        kT = [pers.tile([128, t], f32r, name=f"kT{b}", tag=f"kT{b}") for b in range(B)]
        # v_ext cols: [0:64]=v_j0, 64=ones, [65:129]=v_j1, 129=ones
        v_ext = [pers.tile([128, tt_n, 130], bf16, name=f"vx{b}", tag=f"vx{b}")
                 for b in range(B)]
        wo_sb = [pers.tile([DH, E], bf16, name=f"wo{j}", tag=f"wo{j}") for j in range(2)]
        with tc.high_priority(offset=-150):
            nc.gpsimd.dma_start(out=wo_sb[0], in_=wo0[:, :])
            nc.gpsimd.dma_start(out=wo_sb[1], in_=wo1[:, :])
        exp_bias = pers.tile([128, 1], f32)
        nc.vector.memset(exp_bias, EXP_BIAS)
        for b in range(B):
            nc.vector.memset(v_ext[b][:, :, 64:65], 1.0)
            nc.vector.memset(v_ext[b][:, :, 129:130], 1.0)

        identity = pin.tile([128, 128], f32)
        make_identity(nc, identity)
        wq_sb = pin.tile([128, 8, EC], f32r)
        wk_sb = pin.tile([128, 8, EC], f32r)
        wv_sb = pin.tile([128, 8, EC], f32r)
        nc.sync.dma_start(out=wq_sb, in_=wq[:, :].rearrange("(k p) m -> p k m", p=128))
        nc.sync.dma_start(out=wk_sb, in_=wk[:, :].rearrange("(k p) m -> p k m", p=128))
        nc.sync.dma_start(out=wv_sb, in_=wv[:, :].rearrange("(k p) m -> p k m", p=128))
        bq_sb = pin.tile([128, t], bf16)
        bk_sb = pin.tile([128, t], bf16)
        bv_sb = pin.tile([128, t], bf16)
        with tc.high_priority(offset=-60):
            nc.gpsimd.dma_start(out=bq_sb, in_=bq[:, :])
            nc.gpsimd.dma_start(out=bk_sb, in_=bk[:, :])
            nc.gpsimd.dma_start(out=bv_sb, in_=bv[:, :])

        attnT = {}
        filler = []           # deque of (pe_cost_us, thunk)
        v_done = {0: 0, 1: 0}   # v_ext tiles emitted per batch
        p_ready = {0: True, 1: False}

        def drain_budget(budget):
            while filler and budget > 0:
                c, fn = filler.pop(0)
                fn()
                budget -= c

        def drain_all():
            drain_budget(10 ** 9)

        def need_v(b, st):
            """Force-drain until v_ext[b][:, st] has been emitted (the AV
            matmul that reads it must be emitted after the writer)."""
            while v_done[b] <= st:
                c, fn = filler.pop(0)
                fn()

        def need_p(b):
            while not p_ready[b]:
                c, fn = filler.pop(0)
                fn()

        def proj_chunk(w_sb, b_sb, dst, xt_sb, c4):
            ps = avcpsum.tile([128, 512], f32, tag="avc", name="proj")
            for k in range(8):
                nc.tensor.matmul(
                    ps, lhsT=w_sb[:, k, :], rhs=xt_sb[:, k, ts(c4, 512)],
                    start=(k == 0), stop=(k == 7))
            nc.vector.tensor_add(
                out=dst[:, ts(c4, 512)], in0=ps, in1=b_sb[:, ts(c4, 512)])

        def v_tr_chunk(b, vT, st):
            pt = avcpsum.tile([128, 128], f32, tag="avc", name="tr")
            nc.tensor.transpose(pt, vT[:, ts(st, 128)], identity)
            nc.vector.tensor_copy(out=v_ext[b][:, st, 0:64], in_=pt[:, 0:64])
            nc.vector.tensor_copy(out=v_ext[b][:, st, 65:129], in_=pt[:, 64:128])
            v_done[b] = st + 1

        def stage_p(b, defer=False):
            """Emit projections. defer=True queues everything after the xt
            loads on the filler deque; defer=False emits q/k inline (pass A
            needs them first) and defers only the v chain."""
            xt_sb = pwork.tile([128, 8, t], f32r, tag="xt", name=f"xt{b}")
            vT = pwork.tile([128, t], f32, tag="vT", name=f"vT{b}")
            # Column-chunk-major loads: the first q/k projection chunks (and
            # so the first exps) only need the c4=0 column slice of every
            # k-tile -- 2 MB instead of the full 8 MB. For the deferred
            # batch the loads become filler units too, so the 8 MB trickles
            # in behind the weights-out stream instead of saturating DMA.
            def load_c4(c4):
                for k in range(8):
                    nc.sync.dma_start(
                        out=xt_sb[:, k, ts(c4, 512)],
                        in_=xt[b, ts(k, 128), ts(c4, 512)])
            units = []
            # q/k chunks interleaved so pass A can start earliest
            for c4 in range(q_n):
                units.append((0.5, lambda c4=c4: load_c4(c4)))
                units.append((2.2, lambda c4=c4: proj_chunk(wq_sb, bq_sb, qT[b], xt_sb, c4)))
                units.append((2.2, lambda c4=c4: proj_chunk(wk_sb, bk_sb, kT[b], xt_sb, c4)))
            if not defer:
                for _, u in units:
                    u()
                units = []
            else:
                def mark_ready():
                    p_ready[b] = True
                last_c, last_u = units[-1]
                units[-1] = (last_c, lambda: (last_u(), mark_ready())[0])
            # v chunks with their transposes right behind (tr st needs
            # vT chunk st//4 only)
            for c4 in range(q_n):
                units.append((2.2, lambda c4=c4: proj_chunk(wv_sb, bv_sb, vT, xt_sb, c4)))
                for st in range(c4 * (tt_n // q_n), (c4 + 1) * (tt_n // q_n)):
                    units.append((0.4, lambda st=st: v_tr_chunk(b, vT, st)))
            filler.extend(units)

        def emit_a_tile(b, j, tt):
            qj = qT[b][64 * j:64 * (j + 1), :]
            kj = kT[b][64 * j:64 * (j + 1), :]
            wA = awork.tile([128, t], bf16, tag="wA", name="wA")
            # Alternate the row-sum between ACT's accumulator (187-279ns
            # tax per exp) and a DVE reduce (~2.1us) to balance both engines.
            on_act = tt % 2 == 0
            zs = stats.tile([128, n_ach], f32, tag="z", name="zs") if on_act else None
            for hh in range(n_ach):
                psS = abpsum.tile([128, a_ch], f32, tag="ab", name="psS")
                for sc in range(u_n):
                    nc.tensor.matmul(
                        psS[:, ts(sc, 512)],
                        lhsT=qj[:, ts(tt, 128)],
                        rhs=kj[:, ts(hh * u_n + sc, 512)])
                nc.scalar.activation(
                    wA[:, ts(hh, a_ch)], psS, Exp, bias=exp_bias, scale=1.0,
                    accum_out=zs[:, hh:hh + 1] if on_act else None)
            z = stats.tile([128, 1], f32, tag="zt", name="z")
            if on_act:
                if n_ach == 2:
                    nc.vector.tensor_add(out=z, in0=zs[:, 0:1], in1=zs[:, 1:2])
                else:
                    z = zs[:, 0:1]
            else:
                nc.vector.reduce_sum(out=z, in_=wA, axis=mybir.AxisListType.X)
            rz = stats.tile([128, 1], f32, tag="rz", name="rz")
            nc.vector.reciprocal(rz, z)
            nc.vector.tensor_scalar_mul(wA, in0=wA, scalar1=rz)
            # SWDGE store casts bf16 -> f32 on the way out
            nc.gpsimd.dma_start(out=wout[b, j, ts(tt, 128), :], in_=wA)

        def c_tile(b, tt):
            a0, a1 = attnT[(b, 0)], attnT[(b, 1)]
            po = cwork.tile([128, E], bf16, tag="po", name="po")
            for ech in range(2):
                psO = avcpsum.tile([128, 512], f32, tag="avc", name="psO")
                nc.tensor.matmul(
                    psO, lhsT=a0[:, ts(tt, 128)], rhs=wo_sb[0][:, ts(ech, 512)],
                    start=True, stop=False)
                nc.tensor.matmul(
                    psO, lhsT=a1[:, ts(tt, 128)], rhs=wo_sb[1][:, ts(ech, 512)],
                    start=False, stop=True)
                nc.vector.tensor_copy(out=po[:, ts(ech, 512)], in_=psO)
            nc.sync.dma_start(out=pout[b, ts(tt, 128), :], in_=po)

        def queue_c_half(b, hh):
            for tt in range(hh * (tt_n // n_ach), (hh + 1) * (tt_n // n_ach)):
                filler.append((0.9, lambda tt=tt: c_tile(b, tt)))

        def stage_ab(b, j, budget=0.6, a_dense=False):
            """Pass B slices with pass-A tiles interleaved, so the
            weights-out DMA spreads across the whole (A+B) window. Each
            slice drains up to `budget` us of filler PE work (P/C chunks).
            a_dense packs all A tiles into the first half of the block
            (used for the final block so the tail only holds stage C)."""
            qj = qT[b][64 * j:64 * (j + 1), :]
            kj = kT[b][64 * j:64 * (j + 1), :]
            aT = attnp.tile([DH, t], bf16, name=f"attnT{b}{j}", tag=f"attnT{j}")
            attnT[(b, j)] = aT
            need_p(b)
            a_idx = 0
            for hh in range(n_ach):
                psAV = [avcpsum.tile([65, 512], f32, tag="avc", name=f"psAV{u}")
                        for u in range(u_n)]
                for st in range(tt_n):
                    need_v(b, st)
                    drain_budget(budget)
                    psT = abpsum.tile([128, a_ch], f32, tag="ab", name="psT")
                    for u in range(u_n):
                        nc.tensor.matmul(
                            psT[:, ts(u, 512)], lhsT=kj[:, ts(st, 128)],
                            rhs=qj[:, ts(hh * u_n + u, 512)])
                    wT = bwork.tile([128, a_ch], bf16, tag="wT", name="wT")
                    nc.scalar.activation(wT, psT, Exp, bias=exp_bias, scale=1.0)
                    for u in range(u_n):
                        nc.tensor.matmul(
                            psAV[u], lhsT=v_ext[b][:, st, 65 * j:65 * (j + 1)],
                            rhs=wT[:, ts(u, 512)],
                            start=(st == 0), stop=(st == tt_n - 1))
                    slice_i = hh * tt_n + st
                    a_every = 1 if a_dense else n_ach
                    if a_idx < tt_n and slice_i % a_every == 0:
                        emit_a_tile(b, j, a_idx)
                        a_idx += 1
                for u in range(u_n):
                    h = hh * u_n + u
                    rzT = bstats.tile([1, 512], f32, tag="rzT", name="rzT")
                    nc.vector.reciprocal(rzT, psAV[u][64:65, :])
                    scq = bstats.tile([DH, 512], f32, tag="scq", name="scq")
                    nc.gpsimd.partition_broadcast(scq, rzT)
                    nc.vector.tensor_mul(
                        out=aT[:, ts(h, 512)], in0=psAV[u][0:64, :], in1=scq)
                if j == 1:
                    queue_c_half(b, hh)

        # P and C chunks ride the filler deque, drained by PE-cost budget
        # per B-slice, so their PE/DMA work fills the ACT-bound stretches
        # without monopolizing the shared "avc" psum slots.
        stage_p(0)            # q/k inline; v chain deferred
        stage_p(1, defer=True)
        stage_ab(0, 0)
        stage_ab(0, 1)
        stage_ab(1, 0)
        stage_ab(1, 1, a_dense=True)
        drain_all()

        for p in (cwork, attnp, bstats, stats, bwork, awork, avcpsum, abpsum, pwork, pin, pers):
            p.release()

    nc.finalize()
    return nc


def shard_inputs(query, bias_q, bias_k, bias_v, Wq, Wk, Wv, Wo, t=T):
    """Build per-core input maps (host-side shard + layout prep)."""
    xt = np.ascontiguousarray(query.transpose(1, 2, 0)).astype(np.float32)  # [B, E, T]
    in_maps = []
    for c in range(N_CORES):
        sl = slice(EC * c, EC * (c + 1))
        m = {
            "xt": xt,
            "wq": np.ascontiguousarray((SCALING * Wq[sl, :]).T.astype(np.float32)),
            "wk": np.ascontiguousarray(Wk[sl, :].T.astype(np.float32)),
            "wv": np.ascontiguousarray(Wv[sl, :].T.astype(np.float32)),
            "bq": np.ascontiguousarray((SCALING * bias_q[:, sl]).T.astype(np.float32)),
            "bk": np.ascontiguousarray(bias_k[:, sl].T.astype(np.float32)),
            "bv": np.ascontiguousarray(bias_v[:, sl].T.astype(np.float32)),
            "wo0": np.ascontiguousarray(Wo[:, EC * c: EC * c + DH].T.astype(np.float32)),
            "wo1": np.ascontiguousarray(Wo[:, EC * c + DH: EC * (c + 1)].T.astype(np.float32)),
        }
        in_maps.append(m)
    return in_maps


def assemble_outputs(results, bo, t=T):
    attn_weights = np.empty((B * H, t, t), dtype=np.float32)
    partial = np.zeros((B, t, E), dtype=np.float32)
    for c in range(N_CORES):
        wout = results[c]["wout"]  # [B, 2, t, t]
        for b in range(B):
            for j in range(2):
                attn_weights[b * H + 2 * c + j] = wout[b, j]
        partial += results[c]["pout"]
    attn = partial.transpose(1, 0, 2) + bo[None, None, :].astype(np.float32)
    return np.ascontiguousarray(attn), attn_weights


_NC_CACHE = {}


def kernel(query, key, value, bias_q, bias_k, bias_v, Wq, Wk, Wv, Wo, bo):
    t = query.shape[0]
    if t not in _NC_CACHE:
        _NC_CACHE[t] = build_kernel(t)
    nc = _NC_CACHE[t]
    in_maps = shard_inputs(query, bias_q, bias_k, bias_v, Wq, Wk, Wv, Wo, t=t)
    res = run_bass_kernel_spmd(nc, in_maps, core_ids=list(range(N_CORES)))
    return assemble_outputs(res.results, np.asarray(bo), t=t)


# revision 35
# speedup vs baseline: 1.1376x; 1.0013x over previous
"""FlowAttention kernel for 8 TRN2 NeuronCores.

Sharding: head-parallel. Core c owns heads {2c, 2c+1} of 16, i.e. embed
columns [128c, 128(c+1)).  Each core:
  - projects q/k/v for its 2 heads in transposed layout [dh, T] per batch
    (contraction over E with host-pre-transposed x^T, so no on-chip
    transpose of activations is needed),
  - pass A: scores[t,s] = q^T k, unnormalized exp with fused row-sum
    (ACT accum_out), normalize, write attn_weights slice (the dominant
    64 MB/core output) via SWDGE bf16->f32 cast DMA,
  - pass B: scores^T[s,t] recomputed by swapping matmul operands, exp,
    then attn^T = v_ext^T @ exp(scores^T) where v_ext carries a fused
    ones-column producing the softmax denominator in the transposed
    layout needed for scaling,
  - out-proj partial = attn^T^T @ Wo[:, slice]^T, host sums partials.

PSUM budget (8 banks): "ab" tag 2x[128,1024] (4 banks) shared by pass-A
score chunks, pass-B transposed-score chunks and projection psums;
"avc" tag 4x[128,512] (4 banks) shared by pass-B quarter-accumulators,
stage-C out-proj tiles and stage-P transposes.

Emission order interleaves stage P of batch 1 under pass A of batch 0
so projection DMA/PE work fills the ACT-bound phases.
"""

import numpy as np

import concourse.bass as bass
import concourse.tile as tile
from concourse import bacc, mybir
from concourse.bass import ts
from concourse.bass_utils import run_bass_kernel_spmd
from concourse.masks import make_identity

# Problem constants (hardcoded per harness contract).
T = 2048          # sequence length
B = 2             # batch
E = 1024          # embed dim
H = 16            # heads
DH = 64           # head dim
N_CORES = 8
EC = E // N_CORES  # embed cols per core (= 2 heads * DH = 128)
SCALING = DH ** -0.5
EXP_BIAS = -10.0   # constant shift inside exp; cancels in softmax

f32 = mybir.dt.float32
f32r = mybir.dt.float32r
bf16 = mybir.dt.bfloat16
Exp = mybir.ActivationFunctionType.Exp


def build_kernel(t=T):
    tt_n = t // 128        # 128-tiles along t/s
    q_n = t // 512         # 512-chunks along t/s
    a_ch = min(1024, t)    # score-psum chunk width
    n_ach = t // a_ch
    u_n = a_ch // 512
    nc = bacc.Bacc("TRN2", target_bir_lowering=False)

    # ---- DRAM I/O (per-core shapes) ----
    xt = nc.dram_tensor("xt", [B, E, t], f32r, kind="ExternalInput")
    wq = nc.dram_tensor("wq", [E, EC], f32r, kind="ExternalInput")
    wk = nc.dram_tensor("wk", [E, EC], f32r, kind="ExternalInput")
    wv = nc.dram_tensor("wv", [E, EC], f32r, kind="ExternalInput")
    bq = nc.dram_tensor("bq", [EC, t], f32, kind="ExternalInput")
    bk = nc.dram_tensor("bk", [EC, t], f32, kind="ExternalInput")
    bv = nc.dram_tensor("bv", [EC, t], f32, kind="ExternalInput")
    wo0 = nc.dram_tensor("wo0", [DH, E], f32, kind="ExternalInput")
    wo1 = nc.dram_tensor("wo1", [DH, E], f32, kind="ExternalInput")
    wout = nc.dram_tensor("wout", [B, 2, t, t], f32, kind="ExternalOutput")
    pout = nc.dram_tensor("pout", [B, t, E], bf16, kind="ExternalOutput")

    with tile.TileContext(nc) as tc:
        pers = tc.alloc_tile_pool(name="persist", bufs=1)
        pin = tc.alloc_tile_pool(name="pin", bufs=1)
        pwork = tc.alloc_tile_pool(name="pwork", bufs=1)
        abpsum = tc.alloc_tile_pool(name="abpsum", bufs=2, space="PSUM")
        avcpsum = tc.alloc_tile_pool(name="avcpsum", bufs=4, space="PSUM")
        awork = tc.alloc_tile_pool(name="awork", bufs=7)
        bwork = tc.alloc_tile_pool(name="bwork", bufs=2)
        stats = tc.alloc_tile_pool(name="stats", bufs=4)
        bstats = tc.alloc_tile_pool(name="bstats", bufs=2)
        attnp = tc.alloc_tile_pool(name="attnp", bufs=2)
        cwork = tc.alloc_tile_pool(name="cwork", bufs=2)

        qT = [pers.tile([128, t], f32r, name=f"qT{b}", tag=f"qT{b}") for b in range(B)]
        kT = [pers.tile([128, t], f32r, name=f"kT{b}", tag=f"kT{b}") for b in range(B)]
        # v_ext cols: [0:64]=v_j0, 64=ones, [65:129]=v_j1, 129=ones
        v_ext = [pers.tile([128, tt_n, 130], bf16, name=f"vx{b}", tag=f"vx{b}")
                 for b in range(B)]
        wo_sb = [pers.tile([DH, E], bf16, name=f"wo{j}", tag=f"wo{j}") for j in range(2)]
        with tc.high_priority(offset=-150):
            nc.gpsimd.dma_start(out=wo_sb[0], in_=wo0[:, :])
            nc.gpsimd.dma_start(out=wo_sb[1], in_=wo1[:, :])
        exp_bias = pers.tile([128, 1], f32)
        nc.vector.memset(exp_bias, EXP_BIAS)
        for b in range(B):
            nc.vector.memset(v_ext[b][:, :, 64:65], 1.0)
            nc.vector.memset(v_ext[b][:, :, 129:130], 1.0)

        identity = pin.tile([128, 128], f32)
        make_identity(nc, identity)
        wq_sb = pin.tile([128, 8, EC], f32r)
        wk_sb = pin.tile([128, 8, EC], f32r)
        wv_sb = pin.tile([128, 8, EC], f32r)
        nc.sync.dma_start(out=wq_sb, in_=wq[:, :].rearrange("(k p) m -> p k m", p=128))
        nc.sync.dma_start(out=wk_sb, in_=wk[:, :].rearrange("(k p) m -> p k m", p=128))
        nc.sync.dma_start(out=wv_sb, in_=wv[:, :].rearrange("(k p) m -> p k m", p=128))
        bq_sb = pin.tile([128, t], bf16)
        bk_sb = pin.tile([128, t], bf16)
        bv_sb = pin.tile([128, t], bf16)
        with tc.high_priority(offset=-60):
            nc.gpsimd.dma_start(out=bq_sb, in_=bq[:, :])
            nc.gpsimd.dma_start(out=bk_sb, in_=bk[:, :])
            nc.gpsimd.dma_start(out=bv_sb, in_=bv[:, :])

        attnT = {}
        filler = []           # deque of (pe_cost_us, thunk)
        v_done = {0: 0, 1: 0}   # v_ext tiles emitted per batch
        p_ready = {0: True, 1: False}

        def drain_budget(budget):
            while filler and budget > 0:
                c, fn = filler.pop(0)
                fn()
                budget -= c

        def drain_all():
            drain_budget(10 ** 9)

        def need_v(b, st):
            """Force-drain until v_ext[b][:, st] has been emitted (the AV
            matmul that reads it must be emitted after the writer)."""
            while v_done[b] <= st:
                c, fn = filler.pop(0)
                fn()

        def need_p(b):
            while not p_ready[b]:
                c, fn = filler.pop(0)
                fn()

        def proj_chunk(w_sb, b_sb, dst, xt_sb, c4):
            ps = avcpsum.tile([128, 512], f32, tag="avc", name="proj")
            for k in range(8):
                nc.tensor.matmul(
                    ps, lhsT=w_sb[:, k, :], rhs=xt_sb[:, k, ts(c4, 512)],
                    start=(k == 0), stop=(k == 7))
            nc.vector.tensor_add(
                out=dst[:, ts(c4, 512)], in0=ps, in1=b_sb[:, ts(c4, 512)])

        def v_tr_chunk(b, vT, st):
            pt = avcpsum.tile([128, 128], f32, tag="avc", name="tr")
            nc.tensor.transpose(pt, vT[:, ts(st, 128)], identity)
            nc.vector.tensor_copy(out=v_ext[b][:, st, 0:64], in_=pt[:, 0:64])
            nc.vector.tensor_copy(out=v_ext[b][:, st, 65:129], in_=pt[:, 64:128])
            v_done[b] = st + 1

        def stage_p(b, defer=False):
            """Emit projections. defer=True queues everything after the xt
            loads on the filler deque; defer=False emits q/k inline (pass A
            needs them first) and defers only the v chain."""
            xt_sb = pwork.tile([128, 8, t], f32r, tag="xt", name=f"xt{b}")
            vT = pwork.tile([128, t], f32, tag="vT", name=f"vT{b}")
            # Column-chunk-major loads: the first q/k projection chunks (and
            # so the first exps) only need the c4=0 column slice of every
            # k-tile -- 2 MB instead of the full 8 MB. For the deferred
            # batch the loads become filler units too, so the 8 MB trickles
            # in behind the weights-out stream instead of saturating DMA.
            def load_c4(c4):
                for k in range(8):
                    nc.sync.dma_start(
                        out=xt_sb[:, k, ts(c4, 512)],
                        in_=xt[b, ts(k, 128), ts(c4, 512)])
            units = []
            # q/k chunks interleaved so pass A can start earliest
            for c4 in range(q_n):
                units.append((0.5, lambda c4=c4: load_c4(c4)))
                units.append((2.2, lambda c4=c4: proj_chunk(wq_sb, bq_sb, qT[b], xt_sb, c4)))
                units.append((2.2, lambda c4=c4: proj_chunk(wk_sb, bk_sb, kT[b], xt_sb, c4)))
            if not defer:
                for _, u in units:
                    u()
                units = []
            else:
                def mark_ready():
                    p_ready[b] = True
                last_c, last_u = units[-1]
                units[-1] = (last_c, lambda: (last_u(), mark_ready())[0])
            # v chunks with their transposes right behind (tr st needs
            # vT chunk st//4 only)
            for c4 in range(q_n):
                units.append((2.2, lambda c4=c4: proj_chunk(wv_sb, bv_sb, vT, xt_sb, c4)))
                for st in range(c4 * (tt_n // q_n), (c4 + 1) * (tt_n // q_n)):
                    units.append((0.4, lambda st=st: v_tr_chunk(b, vT, st)))
            filler.extend(units)

        def emit_a_tile(b, j, tt):
            qj = qT[b][64 * j:64 * (j + 1), :]
            kj = kT[b][64 * j:64 * (j + 1), :]
            wA = awork.tile([128, t], bf16, tag="wA", name="wA")
            # Alternate the row-sum between ACT's accumulator (187-279ns
            # tax per exp) and a DVE reduce (~2.1us) to balance both engines.
            on_act = tt % 4 == 0
            zs = stats.tile([128, n_ach], f32, tag="z", name="zs") if on_act else None
            for hh in range(n_ach):
                psS = abpsum.tile([128, a_ch], f32, tag="ab", name="psS")
                for sc in range(u_n):
                    nc.tensor.matmul(
                        psS[:, ts(sc, 512)],
                        lhsT=qj[:, ts(tt, 128)],
                        rhs=kj[:, ts(hh * u_n + sc, 512)])
                nc.scalar.activation(
                    wA[:, ts(hh, a_ch)], psS, Exp, bias=exp_bias, scale=1.0,
                    accum_out=zs[:, hh:hh + 1] if on_act else None)
            z = stats.tile([128, 1], f32, tag="zt", name="z")
            if on_act:
                if n_ach == 2:
                    nc.vector.tensor_add(out=z, in0=zs[:, 0:1], in1=zs[:, 1:2])
                else:
                    z = zs[:, 0:1]
            else:
                nc.vector.reduce_sum(out=z, in_=wA, axis=mybir.AxisListType.X)
            rz = stats.tile([128, 1], f32, tag="rz", name="rz")
            nc.vector.reciprocal(rz, z)
            nc.vector.tensor_scalar_mul(wA, in0=wA, scalar1=rz)
            # SWDGE store casts bf16 -> f32 on the way out
            nc.gpsimd.dma_start(out=wout[b, j, ts(tt, 128), :], in_=wA)

        def c_tile(b, tt):
            a0, a1 = attnT[(b, 0)], attnT[(b, 1)]
            po = cwork.tile([128, E], bf16, tag="po", name="po")
            for ech in range(2):
                psO = avcpsum.tile([128, 512], f32, tag="avc", name="psO")
                nc.tensor.matmul(
                    psO, lhsT=a0[:, ts(tt, 128)], rhs=wo_sb[0][:, ts(ech, 512)],
                    start=True, stop=False)
                nc.tensor.matmul(
                    psO, lhsT=a1[:, ts(tt, 128)], rhs=wo_sb[1][:, ts(ech, 512)],
                    start=False, stop=True)
                nc.vector.tensor_copy(out=po[:, ts(ech, 512)], in_=psO)
            nc.sync.dma_start(out=pout[b, ts(tt, 128), :], in_=po)

        def queue_c_half(b, hh):
            for tt in range(hh * (tt_n // n_ach), (hh + 1) * (tt_n // n_ach)):
                filler.append((0.9, lambda tt=tt: c_tile(b, tt)))

        def stage_ab(b, j, budget=0.6, a_dense=False):
            """Pass B slices with pass-A tiles interleaved, so the
            weights-out DMA spreads across the whole (A+B) window. Each
            slice drains up to `budget` us of filler PE work (P/C chunks).
            a_dense packs all A tiles into the first half of the block
            (used for the final block so the tail only holds stage C)."""
            qj = qT[b][64 * j:64 * (j + 1), :]
            kj = kT[b][64 * j:64 * (j + 1), :]
            aT = attnp.tile([DH, t], bf16, name=f"attnT{b}{j}", tag=f"attnT{j}")
            attnT[(b, j)] = aT
            need_p(b)
            a_idx = 0
            for hh in range(n_ach):
                psAV = [avcpsum.tile([65, 512], f32, tag="avc", name=f"psAV{u}")
                        for u in range(u_n)]
                for st in range(tt_n):
                    need_v(b, st)
                    drain_budget(budget)
                    psT = abpsum.tile([128, a_ch], f32, tag="ab", name="psT")
                    for u in range(u_n):
                        nc.tensor.matmul(
                            psT[:, ts(u, 512)], lhsT=kj[:, ts(st, 128)],
                            rhs=qj[:, ts(hh * u_n + u, 512)])
                    wT = bwork.tile([128, a_ch], bf16, tag="wT", name="wT")
                    nc.scalar.activation(wT, psT, Exp, bias=exp_bias, scale=1.0)
                    for u in range(u_n):
                        nc.tensor.matmul(
                            psAV[u], lhsT=v_ext[b][:, st, 65 * j:65 * (j + 1)],
                            rhs=wT[:, ts(u, 512)],
                            start=(st == 0), stop=(st == tt_n - 1))
                    slice_i = hh * tt_n + st
                    a_every = 1 if a_dense else n_ach
                    if a_idx < tt_n and slice_i % a_every == 0:
                        emit_a_tile(b, j, a_idx)
                        a_idx += 1
                for u in range(u_n):
                    h = hh * u_n + u
                    rzT = bstats.tile([1, 512], f32, tag="rzT", name="rzT")
                    nc.vector.reciprocal(rzT, psAV[u][64:65, :])
                    scq = bstats.tile([DH, 512], f32, tag="scq", name="scq")
                    nc.gpsimd.partition_broadcast(scq, rzT)
                    nc.vector.tensor_mul(
                        out=aT[:, ts(h, 512)], in0=psAV[u][0:64, :], in1=scq)
                    if j == 1:
                        for tt in range(h * 4, h * 4 + 4):
                            filler.append((0.9, lambda tt=tt: c_tile(b, tt)))

        # P and C chunks ride the filler deque, drained by PE-cost budget
        # per B-slice, so their PE/DMA work fills the ACT-bound stretches
        # without monopolizing the shared "avc" psum slots.
        stage_p(0)            # q/k inline; v chain deferred
        stage_p(1, defer=True)
        stage_ab(0, 0)
        stage_ab(0, 1)
        stage_ab(1, 0)
        stage_ab(1, 1, a_dense=True)
        drain_all()

        for p in (cwork, attnp, bstats, stats, bwork, awork, avcpsum, abpsum, pwork, pin, pers):
            p.release()

    nc.finalize()
    return nc


def shard_inputs(query, bias_q, bias_k, bias_v, Wq, Wk, Wv, Wo, t=T):
    """Build per-core input maps (host-side shard + layout prep)."""
    xt = np.ascontiguousarray(query.transpose(1, 2, 0)).astype(np.float32)  # [B, E, T]
    in_maps = []
    for c in range(N_CORES):
        sl = slice(EC * c, EC * (c + 1))
        m = {
            "xt": xt,
            "wq": np.ascontiguousarray((SCALING * Wq[sl, :]).T.astype(np.float32)),
            "wk": np.ascontiguousarray(Wk[sl, :].T.astype(np.float32)),
            "wv": np.ascontiguousarray(Wv[sl, :].T.astype(np.float32)),
            "bq": np.ascontiguousarray((SCALING * bias_q[:, sl]).T.astype(np.float32)),
            "bk": np.ascontiguousarray(bias_k[:, sl].T.astype(np.float32)),
            "bv": np.ascontiguousarray(bias_v[:, sl].T.astype(np.float32)),
            "wo0": np.ascontiguousarray(Wo[:, EC * c: EC * c + DH].T.astype(np.float32)),
            "wo1": np.ascontiguousarray(Wo[:, EC * c + DH: EC * (c + 1)].T.astype(np.float32)),
        }
        in_maps.append(m)
    return in_maps


def assemble_outputs(results, bo, t=T):
    attn_weights = np.empty((B * H, t, t), dtype=np.float32)
    partial = np.zeros((B, t, E), dtype=np.float32)
    for c in range(N_CORES):
        wout = results[c]["wout"]  # [B, 2, t, t]
        for b in range(B):
            for j in range(2):
                attn_weights[b * H + 2 * c + j] = wout[b, j]
        partial += results[c]["pout"]
    attn = partial.transpose(1, 0, 2) + bo[None, None, :].astype(np.float32)
    return np.ascontiguousarray(attn), attn_weights


_NC_CACHE = {}


def kernel(query, key, value, bias_q, bias_k, bias_v, Wq, Wk, Wv, Wo, bo):
    t = query.shape[0]
    if t not in _NC_CACHE:
        _NC_CACHE[t] = build_kernel(t)
    nc = _NC_CACHE[t]
    in_maps = shard_inputs(query, bias_q, bias_k, bias_v, Wq, Wk, Wv, Wo, t=t)
    res = run_bass_kernel_spmd(nc, in_maps, core_ids=list(range(N_CORES)))
    return assemble_outputs(res.results, np.asarray(bo), t=t)


# revision 38
# speedup vs baseline: 1.1728x; 1.0309x over previous
"""FlowAttention kernel for 8 TRN2 NeuronCores.

Sharding: head-parallel. Core c owns heads {2c, 2c+1} of 16, i.e. embed
columns [128c, 128(c+1)).  Each core:
  - projects q/k/v for its 2 heads in transposed layout [dh, T] per batch
    (contraction over E with host-pre-transposed x^T, so no on-chip
    transpose of activations is needed),
  - pass A: scores[t,s] = q^T k, unnormalized exp with fused row-sum
    (ACT accum_out), normalize, write attn_weights slice (the dominant
    64 MB/core output) via SWDGE bf16->f32 cast DMA,
  - pass B: scores^T[s,t] recomputed by swapping matmul operands, exp,
    then attn^T = v_ext^T @ exp(scores^T) where v_ext carries a fused
    ones-column producing the softmax denominator in the transposed
    layout needed for scaling,
  - out-proj partial = attn^T^T @ Wo[:, slice]^T, host sums partials.

PSUM budget (8 banks): "ab" tag 2x[128,1024] (4 banks) shared by pass-A
score chunks, pass-B transposed-score chunks and projection psums;
"avc" tag 4x[128,512] (4 banks) shared by pass-B quarter-accumulators,
stage-C out-proj tiles and stage-P transposes.

Emission order interleaves stage P of batch 1 under pass A of batch 0
so projection DMA/PE work fills the ACT-bound phases.
"""

import numpy as np

import concourse.bass as bass
import concourse.tile as tile
from concourse import bacc, mybir
from concourse.bass import ts
from concourse.bass_utils import run_bass_kernel_spmd
from concourse.masks import make_identity

# Problem constants (hardcoded per harness contract).
T = 2048          # sequence length
B = 2             # batch
E = 1024          # embed dim
H = 16            # heads
DH = 64           # head dim
N_CORES = 8
EC = E // N_CORES  # embed cols per core (= 2 heads * DH = 128)
SCALING = DH ** -0.5
EXP_BIAS = -10.0   # constant shift inside exp; cancels in softmax

f32 = mybir.dt.float32
f32r = mybir.dt.float32r
bf16 = mybir.dt.bfloat16
Exp = mybir.ActivationFunctionType.Exp


def build_kernel(t=T):
    tt_n = t // 128        # 128-tiles along t/s
    q_n = t // 512         # 512-chunks along t/s
    a_ch = min(1024, t)    # score-psum chunk width
    n_ach = t // a_ch
    u_n = a_ch // 512
    nc = bacc.Bacc("TRN2", target_bir_lowering=False)

    # ---- DRAM I/O (per-core shapes) ----
    xt = nc.dram_tensor("xt", [B, E, t], f32r, kind="ExternalInput")
    wq = nc.dram_tensor("wq", [E, EC], f32r, kind="ExternalInput")
    wk = nc.dram_tensor("wk", [E, EC], f32r, kind="ExternalInput")
    wv = nc.dram_tensor("wv", [E, EC], f32r, kind="ExternalInput")
    bq = nc.dram_tensor("bq", [EC, t], f32, kind="ExternalInput")
    bk = nc.dram_tensor("bk", [EC, t], f32, kind="ExternalInput")
    bv = nc.dram_tensor("bv", [EC, t], f32, kind="ExternalInput")
    wo0 = nc.dram_tensor("wo0", [DH, E], f32, kind="ExternalInput")
    wo1 = nc.dram_tensor("wo1", [DH, E], f32, kind="ExternalInput")
    wout = nc.dram_tensor("wout", [B, 2, t, t], f32, kind="ExternalOutput")
    pout = nc.dram_tensor("pout", [B, t, E], bf16, kind="ExternalOutput")

    with tile.TileContext(nc) as tc:
        pers = tc.alloc_tile_pool(name="persist", bufs=1)
        pin = tc.alloc_tile_pool(name="pin", bufs=1)
        pwork = tc.alloc_tile_pool(name="pwork", bufs=1)
        abpsum = tc.alloc_tile_pool(name="abpsum", bufs=2, space="PSUM")
        avcpsum = tc.alloc_tile_pool(name="avcpsum", bufs=4, space="PSUM")
        awork = tc.alloc_tile_pool(name="awork", bufs=7)
        bwork = tc.alloc_tile_pool(name="bwork", bufs=2)
        stats = tc.alloc_tile_pool(name="stats", bufs=4)
        bstats = tc.alloc_tile_pool(name="bstats", bufs=2)
        attnp = tc.alloc_tile_pool(name="attnp", bufs=2)
        cwork = tc.alloc_tile_pool(name="cwork", bufs=2)

        qT = [pers.tile([128, t], f32r, name=f"qT{b}", tag=f"qT{b}") for b in range(B)]
        kT = [pers.tile([128, t], f32r, name=f"kT{b}", tag=f"kT{b}") for b in range(B)]
        # v_ext cols: [0:64]=v_j0, 64=ones, [65:129]=v_j1, 129=ones
        v_ext = [pers.tile([128, tt_n, 130], bf16, name=f"vx{b}", tag=f"vx{b}")
                 for b in range(B)]
        wo_sb = [pers.tile([DH, E], bf16, name=f"wo{j}", tag=f"wo{j}") for j in range(2)]
        with tc.high_priority(offset=-150):
            nc.gpsimd.dma_start(out=wo_sb[0], in_=wo0[:, :])
            nc.gpsimd.dma_start(out=wo_sb[1], in_=wo1[:, :])
        exp_bias = pers.tile([128, 1], f32)
        nc.vector.memset(exp_bias, EXP_BIAS)
        for b in range(B):
            nc.vector.memset(v_ext[b][:, :, 64:65], 1.0)
            nc.vector.memset(v_ext[b][:, :, 129:130], 1.0)

        identity = pin.tile([128, 128], f32)
        make_identity(nc, identity)
        wq_sb = pin.tile([128, 8, EC], f32r)
        wk_sb = pin.tile([128, 8, EC], f32r)
        wv_sb = pin.tile([128, 8, EC], f32r)
        nc.sync.dma_start(out=wq_sb, in_=wq[:, :].rearrange("(k p) m -> p k m", p=128))
        nc.sync.dma_start(out=wk_sb, in_=wk[:, :].rearrange("(k p) m -> p k m", p=128))
        nc.sync.dma_start(out=wv_sb, in_=wv[:, :].rearrange("(k p) m -> p k m", p=128))
        bq_sb = pin.tile([128, t], bf16)
        bk_sb = pin.tile([128, t], bf16)
        bv_sb = pin.tile([128, t], bf16)
        with tc.high_priority(offset=-60):
            nc.gpsimd.dma_start(out=bq_sb, in_=bq[:, :])
            nc.gpsimd.dma_start(out=bk_sb, in_=bk[:, :])
            nc.gpsimd.dma_start(out=bv_sb, in_=bv[:, :])

        attnT = {}
        filler = []           # deque of (pe_cost_us, thunk)
        v_done = {0: 0, 1: 0}   # v_ext tiles emitted per batch
        p_ready = {0: True, 1: False}

        def drain_budget(budget):
            while filler and budget > 0:
                c, fn = filler.pop(0)
                fn()
                budget -= c

        def drain_all():
            drain_budget(10 ** 9)

        def need_v(b, st):
            """Force-drain until v_ext[b][:, st] has been emitted (the AV
            matmul that reads it must be emitted after the writer)."""
            while v_done[b] <= st:
                c, fn = filler.pop(0)
                fn()

        def need_p(b):
            while not p_ready[b]:
                c, fn = filler.pop(0)
                fn()

        def proj_chunk(w_sb, b_sb, dst, xtc, c4):
            ps = avcpsum.tile([128, 512], f32, tag="avc", name="proj")
            for k in range(8):
                nc.tensor.matmul(
                    ps, lhsT=w_sb[:, k, :], rhs=xtc[:, k, :],
                    start=(k == 0), stop=(k == 7))
            nc.vector.tensor_add(
                out=dst[:, ts(c4, 512)], in0=ps, in1=b_sb[:, ts(c4, 512)])

        def v_tr_chunk(b, vT, st):
            pt = avcpsum.tile([128, 128], f32, tag="avc", name="tr")
            nc.tensor.transpose(pt, vT[:, ts(st, 128)], identity)
            nc.vector.tensor_copy(out=v_ext[b][:, st, 0:64], in_=pt[:, 0:64])
            nc.vector.tensor_copy(out=v_ext[b][:, st, 65:129], in_=pt[:, 64:128])
            v_done[b] = st + 1

        def stage_p(b, defer=False):
            """Emit projections. xt lives as four per-column-chunk tiles
            (tag "xt", 4 slots) so each chunk's slot is released as soon as
            its q/k/v projections are done -- the other batch's loads then
            trickle in early instead of arriving as one 8 MB lump.
            defer=True queues everything on the filler deque; defer=False
            emits q/k inline (pass A needs them first), deferring the v
            chain."""
            vT = pwork.tile([128, t], f32, tag="vT", name=f"vT{b}")
            inline, units = [], []
            for c4 in range(q_n):
                xtc = pwork.tile([128, 8, 512], f32r, tag="xt", bufs=4,
                                 name=f"xt{b}c{c4}")

                def load_c4(xtc=xtc, c4=c4):
                    for k in range(8):
                        nc.sync.dma_start(
                            out=xtc[:, k, :],
                            in_=xt[b, ts(k, 128), ts(c4, 512)])
                qk = [
                    (0.5, load_c4),
                    (2.2, lambda xtc=xtc, c4=c4: proj_chunk(wq_sb, bq_sb, qT[b], xtc, c4)),
                    (2.2, lambda xtc=xtc, c4=c4: proj_chunk(wk_sb, bk_sb, kT[b], xtc, c4)),
                ]
                (units if defer else inline).extend(qk)
                # v chunk + its transposes right behind (tr st needs vT
                # chunk st//4 only); last v use releases this xt slot.
                units.append((2.2, lambda xtc=xtc, c4=c4: proj_chunk(wv_sb, bv_sb, vT, xtc, c4)))
                for st in range(c4 * (tt_n // q_n), (c4 + 1) * (tt_n // q_n)):
                    units.append((0.4, lambda st=st: v_tr_chunk(b, vT, st)))
            for _, u in inline:
                u()
            if defer:
                def mark_ready():
                    p_ready[b] = True
                last_c, last_u = units[-1]
                units[-1] = (last_c, lambda: (last_u(), mark_ready())[0])
            filler.extend(units)

        def emit_a_tile(b, j, tt):
            qj = qT[b][64 * j:64 * (j + 1), :]
            kj = kT[b][64 * j:64 * (j + 1), :]
            wA = awork.tile([128, t], bf16, tag="wA", name="wA")
            # Alternate the row-sum between ACT's accumulator (187-279ns
            # tax per exp) and a DVE reduce (~2.1us) to balance both engines.
            on_act = tt % 4 == 0
            zs = stats.tile([128, n_ach], f32, tag="z", name="zs") if on_act else None
            for hh in range(n_ach):
                psS = abpsum.tile([128, a_ch], f32, tag="ab", name="psS")
                for sc in range(u_n):
                    nc.tensor.matmul(
                        psS[:, ts(sc, 512)],
                        lhsT=qj[:, ts(tt, 128)],
                        rhs=kj[:, ts(hh * u_n + sc, 512)])
                nc.scalar.activation(
                    wA[:, ts(hh, a_ch)], psS, Exp, bias=exp_bias, scale=1.0,
                    accum_out=zs[:, hh:hh + 1] if on_act else None)
            z = stats.tile([128, 1], f32, tag="zt", name="z")
            if on_act:
                if n_ach == 2:
                    nc.vector.tensor_add(out=z, in0=zs[:, 0:1], in1=zs[:, 1:2])
                else:
                    z = zs[:, 0:1]
            else:
                nc.vector.reduce_sum(out=z, in_=wA, axis=mybir.AxisListType.X)
            rz = stats.tile([128, 1], f32, tag="rz", name="rz")
            nc.vector.reciprocal(rz, z)
            nc.vector.tensor_scalar_mul(wA, in0=wA, scalar1=rz)
            # SWDGE store casts bf16 -> f32 on the way out
            nc.gpsimd.dma_start(out=wout[b, j, ts(tt, 128), :], in_=wA)

        def c_tile(b, tt):
            a0, a1 = attnT[(b, 0)], attnT[(b, 1)]
            po = cwork.tile([128, E], bf16, tag="po", name="po")
            for ech in range(2):
                psO = avcpsum.tile([128, 512], f32, tag="avc", name="psO")
                nc.tensor.matmul(
                    psO, lhsT=a0[:, ts(tt, 128)], rhs=wo_sb[0][:, ts(ech, 512)],
                    start=True, stop=False)
                nc.tensor.matmul(
                    psO, lhsT=a1[:, ts(tt, 128)], rhs=wo_sb[1][:, ts(ech, 512)],
                    start=False, stop=True)
                nc.vector.tensor_copy(out=po[:, ts(ech, 512)], in_=psO)
            nc.sync.dma_start(out=pout[b, ts(tt, 128), :], in_=po)

        def queue_c_half(b, hh):
            for tt in range(hh * (tt_n // n_ach), (hh + 1) * (tt_n // n_ach)):
                filler.append((0.9, lambda tt=tt: c_tile(b, tt)))

        def stage_ab(b, j, budget=0.6, a_dense=False):
            """Pass B slices with pass-A tiles interleaved, so the
            weights-out DMA spreads across the whole (A+B) window. Each
            slice drains up to `budget` us of filler PE work (P/C chunks).
            a_dense packs all A tiles into the first half of the block
            (used for the final block so the tail only holds stage C)."""
            qj = qT[b][64 * j:64 * (j + 1), :]
            kj = kT[b][64 * j:64 * (j + 1), :]
            aT = attnp.tile([DH, t], bf16, name=f"attnT{b}{j}", tag=f"attnT{j}")
            attnT[(b, j)] = aT
            need_p(b)
            a_idx = 0
            for hh in range(n_ach):
                psAV = [avcpsum.tile([65, 512], f32, tag="avc", name=f"psAV{u}")
                        for u in range(u_n)]
                for st in range(tt_n):
                    need_v(b, st)
                    drain_budget(budget)
                    psT = abpsum.tile([128, a_ch], f32, tag="ab", name="psT")
                    for u in range(u_n):
                        nc.tensor.matmul(
                            psT[:, ts(u, 512)], lhsT=kj[:, ts(st, 128)],
                            rhs=qj[:, ts(hh * u_n + u, 512)])
                    wT = bwork.tile([128, a_ch], bf16, tag="wT", name="wT")
                    nc.scalar.activation(wT, psT, Exp, bias=exp_bias, scale=1.0)
                    for u in range(u_n):
                        nc.tensor.matmul(
                            psAV[u], lhsT=v_ext[b][:, st, 65 * j:65 * (j + 1)],
                            rhs=wT[:, ts(u, 512)],
                            start=(st == 0), stop=(st == tt_n - 1))
                    slice_i = hh * tt_n + st
                    a_every = 1 if a_dense else n_ach
                    if a_idx < tt_n and slice_i % a_every == 0:
                        emit_a_tile(b, j, a_idx)
                        a_idx += 1
                for u in range(u_n):
                    h = hh * u_n + u
                    rzT = bstats.tile([1, 512], f32, tag="rzT", name="rzT")
                    nc.vector.reciprocal(rzT, psAV[u][64:65, :])
                    scq = bstats.tile([DH, 512], f32, tag="scq", name="scq")
                    nc.gpsimd.partition_broadcast(scq, rzT)
                    nc.vector.tensor_mul(
                        out=aT[:, ts(h, 512)], in0=psAV[u][0:64, :], in1=scq)
                    if j == 1:
                        for tt in range(h * 4, h * 4 + 4):
                            filler.append((0.9, lambda tt=tt: c_tile(b, tt)))

        # P and C chunks ride the filler deque, drained by PE-cost budget
        # per B-slice, so their PE/DMA work fills the ACT-bound stretches
        # without monopolizing the shared "avc" psum slots.
        stage_p(0)            # q/k inline; v chain deferred
        stage_p(1, defer=True)
        stage_ab(0, 0)
        stage_ab(0, 1)
        stage_ab(1, 0)
        stage_ab(1, 1, a_dense=True)
        drain_all()

        for p in (cwork, attnp, bstats, stats, bwork, awork, avcpsum, abpsum, pwork, pin, pers):
            p.release()

    nc.finalize()
    return nc


def shard_inputs(query, bias_q, bias_k, bias_v, Wq, Wk, Wv, Wo, t=T):
    """Build per-core input maps (host-side shard + layout prep)."""
    xt = np.ascontiguousarray(query.transpose(1, 2, 0)).astype(np.float32)  # [B, E, T]
    in_maps = []
    for c in range(N_CORES):
        sl = slice(EC * c, EC * (c + 1))
        m = {
            "xt": xt,
            "wq": np.ascontiguousarray((SCALING * Wq[sl, :]).T.astype(np.float32)),
            "wk": np.ascontiguousarray(Wk[sl, :].T.astype(np.float32)),
            "wv": np.ascontiguousarray(Wv[sl, :].T.astype(np.float32)),
            "bq": np.ascontiguousarray((SCALING * bias_q[:, sl]).T.astype(np.float32)),
            "bk": np.ascontiguousarray(bias_k[:, sl].T.astype(np.float32)),
            "bv": np.ascontiguousarray(bias_v[:, sl].T.astype(np.float32)),
            "wo0": np.ascontiguousarray(Wo[:, EC * c: EC * c + DH].T.astype(np.float32)),
            "wo1": np.ascontiguousarray(Wo[:, EC * c + DH: EC * (c + 1)].T.astype(np.float32)),
        }
        in_maps.append(m)
    return in_maps


def assemble_outputs(results, bo, t=T):
    attn_weights = np.empty((B * H, t, t), dtype=np.float32)
    partial = np.zeros((B, t, E), dtype=np.float32)
    for c in range(N_CORES):
        wout = results[c]["wout"]  # [B, 2, t, t]
        for b in range(B):
            for j in range(2):
                attn_weights[b * H + 2 * c + j] = wout[b, j]
        partial += results[c]["pout"]
    attn = partial.transpose(1, 0, 2) + bo[None, None, :].astype(np.float32)
    return np.ascontiguousarray(attn), attn_weights


_NC_CACHE = {}


def kernel(query, key, value, bias_q, bias_k, bias_v, Wq, Wk, Wv, Wo, bo):
    t = query.shape[0]
    if t not in _NC_CACHE:
        _NC_CACHE[t] = build_kernel(t)
    nc = _NC_CACHE[t]
    in_maps = shard_inputs(query, bias_q, bias_k, bias_v, Wq, Wk, Wv, Wo, t=t)
    res = run_bass_kernel_spmd(nc, in_maps, core_ids=list(range(N_CORES)))
    return assemble_outputs(res.results, np.asarray(bo), t=t)


# revision 45
# speedup vs baseline: 1.1842x; 1.0097x over previous
"""FlowAttention kernel for 8 TRN2 NeuronCores.

Sharding: head-parallel. Core c owns heads {2c, 2c+1} of 16, i.e. embed
columns [128c, 128(c+1)).  Each core:
  - projects q/k/v for its 2 heads in transposed layout [dh, T] per batch
    (contraction over E with host-pre-transposed x^T, so no on-chip
    transpose of activations is needed),
  - pass A: scores[t,s] = q^T k, unnormalized exp with fused row-sum
    (ACT accum_out), normalize, write attn_weights slice (the dominant
    64 MB/core output) via SWDGE bf16->f32 cast DMA,
  - pass B: scores^T[s,t] recomputed by swapping matmul operands, exp,
    then attn^T = v_ext^T @ exp(scores^T) where v_ext carries a fused
    ones-column producing the softmax denominator in the transposed
    layout needed for scaling,
  - out-proj partial = attn^T^T @ Wo[:, slice]^T, host sums partials.

PSUM budget (8 banks): "ab" tag 2x[128,1024] (4 banks) shared by pass-A
score chunks, pass-B transposed-score chunks and projection psums;
"avc" tag 4x[128,512] (4 banks) shared by pass-B quarter-accumulators,
stage-C out-proj tiles and stage-P transposes.

Emission order interleaves stage P of batch 1 under pass A of batch 0
so projection DMA/PE work fills the ACT-bound phases.
"""

import numpy as np

import concourse.bass as bass
import concourse.tile as tile
from concourse import bacc, mybir
from concourse.bass import ts
from concourse.bass_utils import run_bass_kernel_spmd
from concourse.masks import make_identity

# Problem constants (hardcoded per harness contract).
T = 2048          # sequence length
B = 2             # batch
E = 1024          # embed dim
H = 16            # heads
DH = 64           # head dim
N_CORES = 8
EC = E // N_CORES  # embed cols per core (= 2 heads * DH = 128)
SCALING = DH ** -0.5
EXP_BIAS = -10.0   # constant shift inside exp; cancels in softmax

f32 = mybir.dt.float32
f32r = mybir.dt.float32r
bf16 = mybir.dt.bfloat16
Exp = mybir.ActivationFunctionType.Exp


def build_kernel(t=T):
    tt_n = t // 128        # 128-tiles along t/s
    q_n = t // 512         # 512-chunks along t/s
    a_ch = min(1024, t)    # score-psum chunk width
    n_ach = t // a_ch
    u_n = a_ch // 512
    nc = bacc.Bacc("TRN2", target_bir_lowering=False)

    # ---- DRAM I/O (per-core shapes) ----
    xt = nc.dram_tensor("xt", [B, E, t], f32r, kind="ExternalInput")
    wq = nc.dram_tensor("wq", [E, EC], f32r, kind="ExternalInput")
    wk = nc.dram_tensor("wk", [E, EC], f32r, kind="ExternalInput")
    wv = nc.dram_tensor("wv", [E, EC], f32r, kind="ExternalInput")
    bq = nc.dram_tensor("bq", [EC, t], f32, kind="ExternalInput")
    bk = nc.dram_tensor("bk", [EC, t], f32, kind="ExternalInput")
    bv = nc.dram_tensor("bv", [EC, t], f32, kind="ExternalInput")
    wo0 = nc.dram_tensor("wo0", [DH, E], f32, kind="ExternalInput")
    wo1 = nc.dram_tensor("wo1", [DH, E], f32, kind="ExternalInput")
    wout = nc.dram_tensor("wout", [B, 2, t, t], f32, kind="ExternalOutput")
    pout = nc.dram_tensor("pout", [B, t, E], bf16, kind="ExternalOutput")

    with tile.TileContext(nc) as tc:
        pers = tc.alloc_tile_pool(name="persist", bufs=1)
        pin = tc.alloc_tile_pool(name="pin", bufs=1)
        pwork = tc.alloc_tile_pool(name="pwork", bufs=1)
        abpsum = tc.alloc_tile_pool(name="abpsum", bufs=2, space="PSUM")
        avcpsum = tc.alloc_tile_pool(name="avcpsum", bufs=4, space="PSUM")
        awork = tc.alloc_tile_pool(name="awork", bufs=7)
        bwork = tc.alloc_tile_pool(name="bwork", bufs=2)
        stats = tc.alloc_tile_pool(name="stats", bufs=4)
        bstats = tc.alloc_tile_pool(name="bstats", bufs=2)
        attnp = tc.alloc_tile_pool(name="attnp", bufs=2)
        cwork = tc.alloc_tile_pool(name="cwork", bufs=2)

        qT = [pers.tile([128, t], f32r, name=f"qT{b}", tag=f"qT{b}") for b in range(B)]
        kT = [pers.tile([128, t], f32r, name=f"kT{b}", tag=f"kT{b}") for b in range(B)]
        # v_ext cols: [0:64]=v_j0, 64=ones, [65:129]=v_j1, 129=ones
        v_ext = [pers.tile([128, tt_n, 130], bf16, name=f"vx{b}", tag=f"vx{b}")
                 for b in range(B)]
        wo_sb = [pers.tile([DH, E], bf16, name=f"wo{j}", tag=f"wo{j}") for j in range(2)]
        with tc.high_priority(offset=-150):
            nc.gpsimd.dma_start(out=wo_sb[0], in_=wo0[:, :])
            nc.gpsimd.dma_start(out=wo_sb[1], in_=wo1[:, :])
        exp_bias = pers.tile([128, 1], f32)
        nc.vector.memset(exp_bias, EXP_BIAS)
        for b in range(B):
            nc.vector.memset(v_ext[b][:, :, 64:65], 1.0)
            nc.vector.memset(v_ext[b][:, :, 129:130], 1.0)

        identity = pin.tile([128, 128], f32)
        make_identity(nc, identity)
        wq_sb = pin.tile([128, 8, EC], f32r)
        wk_sb = pin.tile([128, 8, EC], f32r)
        wv_sb = pin.tile([128, 8, EC], f32r)
        nc.sync.dma_start(out=wq_sb, in_=wq[:, :].rearrange("(k p) m -> p k m", p=128))
        nc.sync.dma_start(out=wk_sb, in_=wk[:, :].rearrange("(k p) m -> p k m", p=128))
        nc.sync.dma_start(out=wv_sb, in_=wv[:, :].rearrange("(k p) m -> p k m", p=128))
        bq_sb = pin.tile([128, t], bf16)
        bk_sb = pin.tile([128, t], bf16)
        bv_sb = pin.tile([128, t], bf16)
        with tc.high_priority(offset=-60):
            nc.gpsimd.dma_start(out=bq_sb, in_=bq[:, :])
            nc.gpsimd.dma_start(out=bk_sb, in_=bk[:, :])
            nc.gpsimd.dma_start(out=bv_sb, in_=bv[:, :])

        attnT = {}
        filler = []           # deque of (pe_cost_us, thunk)
        v_done = {0: 0, 1: 0}   # v_ext tiles emitted per batch
        p_ready = {0: True, 1: False}

        def drain_budget(budget):
            while filler and budget > 0:
                c, fn = filler.pop(0)
                fn()
                budget -= c

        def drain_all():
            drain_budget(10 ** 9)

        def need_v(b, st):
            """Force-drain until v_ext[b][:, st] has been emitted (the AV
            matmul that reads it must be emitted after the writer)."""
            while v_done[b] <= st:
                c, fn = filler.pop(0)
                fn()

        def need_p(b):
            while not p_ready[b]:
                c, fn = filler.pop(0)
                fn()

        def proj_chunk(w_sb, b_sb, dst, xtc, c4):
            ps = avcpsum.tile([128, 512], f32, tag="avc", name="proj")
            for k in range(8):
                nc.tensor.matmul(
                    ps, lhsT=w_sb[:, k, :], rhs=xtc[:, k, :],
                    start=(k == 0), stop=(k == 7))
            nc.vector.tensor_add(
                out=dst[:, ts(c4, 512)], in0=ps, in1=b_sb[:, ts(c4, 512)])

        def v_tr_chunk(b, vT, st):
            pt = avcpsum.tile([128, 128], f32, tag="avc", name="tr")
            nc.tensor.transpose(pt, vT[:, ts(st, 128)], identity)
            nc.vector.tensor_copy(out=v_ext[b][:, st, 0:64], in_=pt[:, 0:64])
            nc.vector.tensor_copy(out=v_ext[b][:, st, 65:129], in_=pt[:, 64:128])
            v_done[b] = st + 1

        def stage_p(b, defer=False):
            """Emit projections. xt lives as four per-column-chunk tiles
            (tag "xt", 4 slots) so each chunk's slot is released as soon as
            its q/k/v projections are done -- the other batch's loads then
            trickle in early instead of arriving as one 8 MB lump.
            defer=True queues everything on the filler deque; defer=False
            emits q/k inline (pass A needs them first), deferring the v
            chain."""
            vT = pwork.tile([128, t], f32, tag="vT", name=f"vT{b}")
            inline, units = [], []
            for c4 in range(q_n):
                xtc = pwork.tile([128, 8, 512], f32r, tag="xt", bufs=4,
                                 name=f"xt{b}c{c4}")

                def load_c4(xtc=xtc, c4=c4):
                    for k in range(8):
                        nc.sync.dma_start(
                            out=xtc[:, k, :],
                            in_=xt[b, ts(k, 128), ts(c4, 512)])
                qk = [
                    (0.5, load_c4),
                    (2.2, lambda xtc=xtc, c4=c4: proj_chunk(wq_sb, bq_sb, qT[b], xtc, c4)),
                    (2.2, lambda xtc=xtc, c4=c4: proj_chunk(wk_sb, bk_sb, kT[b], xtc, c4)),
                ]
                (units if defer else inline).extend(qk)
                # v chunk + its transposes right behind (tr st needs vT
                # chunk st//4 only); last v use releases this xt slot.
                units.append((2.2, lambda xtc=xtc, c4=c4: proj_chunk(wv_sb, bv_sb, vT, xtc, c4)))
                for st in range(c4 * (tt_n // q_n), (c4 + 1) * (tt_n // q_n)):
                    units.append((0.4, lambda st=st: v_tr_chunk(b, vT, st)))
            for _, u in inline:
                u()
            if defer:
                def mark_ready():
                    p_ready[b] = True
                last_c, last_u = units[-1]
                units[-1] = (last_c, lambda: (last_u(), mark_ready())[0])
            filler.extend(units)

        def emit_a_tile(b, j, tt):
            qj = qT[b][64 * j:64 * (j + 1), :]
            kj = kT[b][64 * j:64 * (j + 1), :]
            wA = awork.tile([128, t], bf16, tag="wA", name="wA")
            # Alternate the row-sum between ACT's accumulator (187-279ns
            # tax per exp) and a DVE reduce (~2.1us) to balance both engines.
            on_act = tt % 4 == 0
            zs = stats.tile([128, n_ach], f32, tag="z", name="zs") if on_act else None
            for hh in range(n_ach):
                psS = abpsum.tile([128, a_ch], f32, tag="ab", name="psS")
                for sc in range(u_n):
                    nc.tensor.matmul(
                        psS[:, ts(sc, 512)],
                        lhsT=qj[:, ts(tt, 128)],
                        rhs=kj[:, ts(hh * u_n + sc, 512)])
                nc.scalar.activation(
                    wA[:, ts(hh, a_ch)], psS, Exp, bias=exp_bias, scale=1.0,
                    accum_out=zs[:, hh:hh + 1] if on_act else None)
            z = stats.tile([128, 1], f32, tag="zt", name="z")
            if on_act:
                if n_ach == 2:
                    nc.vector.tensor_add(out=z, in0=zs[:, 0:1], in1=zs[:, 1:2])
                else:
                    z = zs[:, 0:1]
            else:
                nc.vector.reduce_sum(out=z, in_=wA, axis=mybir.AxisListType.X)
            rz = stats.tile([128, 1], f32, tag="rz", name="rz")
            nc.vector.reciprocal(rz, z)
            nc.vector.tensor_scalar_mul(wA, in0=wA, scalar1=rz)
            # SWDGE store casts bf16 -> f32 on the way out
            nc.gpsimd.dma_start(out=wout[b, j, ts(tt, 128), :], in_=wA)

        def c_tile(b, tt):
            a0, a1 = attnT[(b, 0)], attnT[(b, 1)]
            po = cwork.tile([128, E], bf16, tag="po", name="po")
            for ech in range(2):
                psO = avcpsum.tile([128, 512], f32, tag="avc", name="psO")
                nc.tensor.matmul(
                    psO, lhsT=a0[:, ts(tt, 128)], rhs=wo_sb[0][:, ts(ech, 512)],
                    start=True, stop=False)
                nc.tensor.matmul(
                    psO, lhsT=a1[:, ts(tt, 128)], rhs=wo_sb[1][:, ts(ech, 512)],
                    start=False, stop=True)
                nc.vector.tensor_copy(out=po[:, ts(ech, 512)], in_=psO)
            nc.sync.dma_start(out=pout[b, ts(tt, 128), :], in_=po)

        def queue_c_half(b, hh):
            for tt in range(hh * (tt_n // n_ach), (hh + 1) * (tt_n // n_ach)):
                filler.append((0.9, lambda tt=tt: c_tile(b, tt)))

        def stage_ab(b, j, budget=0.6, a_dense=False):
            """Pass B slices with pass-A tiles interleaved, so the
            weights-out DMA spreads across the whole (A+B) window. Each
            slice drains up to `budget` us of filler PE work (P/C chunks).
            a_dense packs all A tiles into the first half of the block
            (used for the final block so the tail only holds stage C)."""
            qj = qT[b][64 * j:64 * (j + 1), :]
            kj = kT[b][64 * j:64 * (j + 1), :]
            aT = attnp.tile([DH, t], bf16, name=f"attnT{b}{j}", tag=f"attnT{j}")
            attnT[(b, j)] = aT
            need_p(b)
            a_idx = 0
            for hh in range(n_ach):
                psAV = [avcpsum.tile([65, 512], f32, tag="avc", name=f"psAV{u}")
                        for u in range(u_n)]
                for st in range(tt_n):
                    need_v(b, st)
                    drain_budget(budget)
                    psT = abpsum.tile([128, a_ch], f32, tag="ab", name="psT")
                    for u in range(u_n):
                        nc.tensor.matmul(
                            psT[:, ts(u, 512)], lhsT=kj[:, ts(st, 128)],
                            rhs=qj[:, ts(hh * u_n + u, 512)])
                    wT = bwork.tile([128, a_ch], bf16, tag="wT", name="wT")
                    nc.scalar.activation(wT, psT, Exp, bias=exp_bias, scale=1.0)
                    for u in range(u_n):
                        nc.tensor.matmul(
                            psAV[u], lhsT=v_ext[b][:, st, 65 * j:65 * (j + 1)],
                            rhs=wT[:, ts(u, 512)],
                            start=(st == 0), stop=(st == tt_n - 1))
                    slice_i = hh * tt_n + st
                    a_every = 1 if a_dense else n_ach
                    if a_idx < tt_n and slice_i % a_every == 0:
                        emit_a_tile(b, j, a_idx)
                        a_idx += 1
                for u in range(u_n):
                    h = hh * u_n + u
                    rzT = bstats.tile([1, 512], f32, tag="rzT", name="rzT")
                    nc.vector.reciprocal(rzT, psAV[u][64:65, :])
                    scq = bstats.tile([DH, 512], f32, tag="scq", name="scq")
                    nc.gpsimd.partition_broadcast(scq, rzT)
                    nc.vector.tensor_mul(
                        out=aT[:, ts(h, 512)], in0=psAV[u][0:64, :], in1=scq)
                    if j == 1:
                        for tt in range(h * 4, h * 4 + 4):
                            filler.append((0.9, lambda tt=tt: c_tile(b, tt)))

        # P and C chunks ride the filler deque, drained by PE-cost budget
        # per B-slice, so their PE/DMA work fills the ACT-bound stretches
        # without monopolizing the shared "avc" psum slots.
        stage_p(0)            # q/k inline; v chain deferred
        stage_p(1, defer=True)
        stage_ab(0, 0)
        stage_ab(0, 1)
        stage_ab(1, 0, a_dense=True)
        stage_ab(1, 1, a_dense=True)
        drain_all()

        for p in (cwork, attnp, bstats, stats, bwork, awork, avcpsum, abpsum, pwork, pin, pers):
            p.release()

    nc.finalize()
    return nc


def shard_inputs(query, bias_q, bias_k, bias_v, Wq, Wk, Wv, Wo, t=T):
    """Build per-core input maps (host-side shard + layout prep)."""
    xt = np.ascontiguousarray(query.transpose(1, 2, 0)).astype(np.float32)  # [B, E, T]
    in_maps = []
    for c in range(N_CORES):
        sl = slice(EC * c, EC * (c + 1))
        m = {
            "xt": xt,
            "wq": np.ascontiguousarray((SCALING * Wq[sl, :]).T.astype(np.float32)),
            "wk": np.ascontiguousarray(Wk[sl, :].T.astype(np.float32)),
            "wv": np.ascontiguousarray(Wv[sl, :].T.astype(np.float32)),
            "bq": np.ascontiguousarray((SCALING * bias_q[:, sl]).T.astype(np.float32)),
            "bk": np.ascontiguousarray(bias_k[:, sl].T.astype(np.float32)),
            "bv": np.ascontiguousarray(bias_v[:, sl].T.astype(np.float32)),
            "wo0": np.ascontiguousarray(Wo[:, EC * c: EC * c + DH].T.astype(np.float32)),
            "wo1": np.ascontiguousarray(Wo[:, EC * c + DH: EC * (c + 1)].T.astype(np.float32)),
        }
        in_maps.append(m)
    return in_maps


def assemble_outputs(results, bo, t=T):
    attn_weights = np.empty((B * H, t, t), dtype=np.float32)
    partial = np.zeros((B, t, E), dtype=np.float32)
    for c in range(N_CORES):
        wout = results[c]["wout"]  # [B, 2, t, t]
        for b in range(B):
            for j in range(2):
                attn_weights[b * H + 2 * c + j] = wout[b, j]
        partial += results[c]["pout"]
    attn = partial.transpose(1, 0, 2) + bo[None, None, :].astype(np.float32)
    return np.ascontiguousarray(attn), attn_weights


_NC_CACHE = {}


def kernel(query, key, value, bias_q, bias_k, bias_v, Wq, Wk, Wv, Wo, bo):
    t = query.shape[0]
    if t not in _NC_CACHE:
        _NC_CACHE[t] = build_kernel(t)
    nc = _NC_CACHE[t]
    in_maps = shard_inputs(query, bias_q, bias_k, bias_v, Wq, Wk, Wv, Wo, t=t)
    res = run_bass_kernel_spmd(nc, in_maps, core_ids=list(range(N_CORES)))
    return assemble_outputs(res.results, np.asarray(bo), t=t)
